# revision 28
# baseline (speedup 1.0000x reference)
"""Multi-head attention Bass kernel for Trainium2, sharded over 8 NeuronCores.

Sharding: core c handles batch b = c//4 and head-group g = c%4 (4 of 16 heads,
i.e. a 256-wide slice of the QKV projection output).  Each core computes its
heads' attention and a partial output projection (contribution of its 256
ctx columns to the full [S, D] output).  The host sums the 4 partials per
batch (fp32) and adds the output bias.

Device-side design (v4 — ScalarE-saturated pipeline, HAM kept warm):
  - activations shipped pre-transposed: xT = x.T  [D, S]; weights shipped
    pre-packed in the exact SBUF layout (contiguous DMA).
  - scores computed transposed (scoresT[sk, sq]) so attention weights leave
    softmax with sk on partitions — the contraction layout attn@V needs.
  - softmax denominator comes free from a ones-column appended to V
    (ctx psum row 64 = sum_sk attn);  no max-subtraction (scores bounded).
  - stage B processes one head at a time, sweeping sk chunks with a
    double-buffered score psum: the PE issues scores(sk+2) the moment
    exp(sk) frees a buffer, so the Exp stream on ScalarE (the critical
    engine: ~1 elem/cycle/lane) never waits.  ctx matmuls trail one step
    so psum hand-offs can't head-of-line-block the PE queue.
  - HAM (PE clock gate) management: dummy warm-up matmuls cover the DMA
    latency head; the Q projection is split — its hm=1 half is injected
    as filler matmuls into the first sweep so the PE has no idle window
    at the stage A->B transition and stays at 2.4 GHz.
  - normalize frees ctx-psum with one staging copy and runs the
    recip/broadcast/divide lazily; even heads write straight into ctxT.
  - output projection is emitted one unit per sweep step into the PE's
    idle slack during the NEXT sq block's sweeps; final block drains in a
    short tail with psum->SBUF copies split across Vector/Scalar.
  - psum budget (16KB/part): score 2x[128,1024]f32 + ctx 1x[128,1024]f32
    + oproj 2x[128,512]f32 = 8 banks exactly.  Stage A reuses all four
    pools as 8 independent 512-wide accumulation slots, kk-outer.
"""

import numpy as np
import ml_dtypes

import concourse.bass as bass
import concourse.mybir as mybir
import concourse.tile as tile
from concourse import bacc, library_config
from concourse.bass_utils import run_bass_kernel_spmd

# Problem shapes (hardcoded per contest rules).
B, S, D, H, DH = 2, 2048, 1024, 16, 64
NCORES = 8
NH = 4            # heads per core
DQ = NH * DH      # 256: per-core q/k/v width
P = 128

F32 = mybir.dt.float32
F16 = mybir.dt.float16
BF16 = mybir.dt.bfloat16
NP_BF16 = ml_dtypes.bfloat16

SQC = 1024        # sq block per sweep
NSQ = S // SQC    # 2
SKN = S // P      # 16 sk chunks
KO = D // P       # 8 contraction chunks for projections
MQ = DQ // P      # 2


def build_nc():
    """Build the per-core Bass program (same NEFF on all 8 cores)."""
    nc = bacc.Bacc("TRN2", debug=False)

    xq_t = nc.declare_dram_parameter("xq", [D, S], BF16, isOutput=False)
    xk_t = nc.declare_dram_parameter("xk", [D, S], BF16, isOutput=False)
    xv_t = nc.declare_dram_parameter("xv", [D, S], BF16, isOutput=False)
    wq_t = nc.declare_dram_parameter("wq", [P, KO * DQ], BF16, isOutput=False)
    wk_t = nc.declare_dram_parameter("wk", [P, KO * DQ], BF16, isOutput=False)
    wv_t = nc.declare_dram_parameter("wv", [P, KO * DQ], BF16, isOutput=False)
    wo_t = nc.declare_dram_parameter("wo", [P, MQ * D], BF16, isOutput=False)
    bq_t = nc.declare_dram_parameter("bq", [P, MQ], F32, isOutput=False)
    bk_t = nc.declare_dram_parameter("bk", [P, MQ], F32, isOutput=False)
    bv_t = nc.declare_dram_parameter("bv", [P, DQ], F32, isOutput=False)
    keep_t = nc.declare_dram_parameter("keep", [S, S], BF16, isOutput=False)
    out_t = nc.declare_dram_parameter("out", [D, S], F16, isOutput=True)

    AF = mybir.ActivationFunctionType
    OP = mybir.AluOpType

    with tile.TileContext(nc) as tc:
        nc.gpsimd.load_library(library_config.attn)
        with (
            tc.tile_pool(name="const", bufs=1) as const,
            tc.tile_pool(name="xs", bufs=3) as xs,
            tc.tile_pool(name="attn", bufs=9) as attnp,
            tc.tile_pool(name="sc", bufs=2) as scp,
            tc.tile_pool(name="outp", bufs=3) as outp,
            tc.tile_pool(name="ps_s", bufs=2, space="PSUM") as ps_s,
            tc.tile_pool(name="ps_c", bufs=1, space="PSUM") as ps_c,
            tc.tile_pool(name="ps_o", bufs=2, space="PSUM") as ps_o,
        ):
            # ---- persistent SBUF tensors ----
            wq_sb = const.tile([P, KO, DQ], BF16, tag="wq")
            wk_sb = const.tile([P, KO, DQ], BF16, tag="wk")
            wv_sb = const.tile([P, KO, DQ], BF16, tag="wv")
            wo_sb = const.tile([P, MQ, D], BF16, tag="wo")
            bq_sb = const.tile([P, MQ], F32, tag="bq")
            bk_sb = const.tile([P, MQ], F32, tag="bk")
            bv_sb = const.tile([P, DQ], F32, tag="bv")
            qT_sb = const.tile([P, MQ, S], BF16, tag="qT")
            kT_sb = const.tile([P, MQ, S], BF16, tag="kT")
            v_sb = const.tile([P, SKN, NH * 65], BF16, tag="v")
            keep_sb = const.tile([P, SKN, S], BF16, tag="keep")
            ctxT_sb = const.tile([P, MQ, S], BF16, tag="ctxT")
            xa_sb = const.tile([P, KO, S], BF16, tag="xa")  # resident xv->xq
            warm = const.tile([1, 8], F32, tag="warm")
            wmm = const.tile([1, 512], BF16, tag="wmm")

            # preload the exp table set on ScalarE while stage A runs
            nc.vector.memset(warm, 0.0)
            nc.scalar.activation(warm, warm, AF.Exp, scale=1.0)
            nc.vector.memset(wmm, 0.0)

            pswarm = ps_o.tile([P, 512], F32, tag="o")

            def warm_burst(lhs, rhs, n=10, dst=None):
                """Dummy matmuls to keep the PE HAM clock-gate open."""
                w = rhs.shape[-1]
                if dst is None:
                    dst = pswarm
                for _ in range(n):
                    nc.tensor.matmul(dst[0:64, 0:w], lhs,
                                     rhs, start=True, stop=True)

            warm_burst(wmm[0:1, 0:64], wmm[0:1, 0:512], 16)

            # ones column per head in the V tile (softmax denominator trick)
            v_strided = v_sb[:].rearrange("p s (h c) -> p s h c", h=NH)
            nc.vector.memset(v_strided[:, :, :, 64:65], 1.0)

            def psum_slots():
                """8 independent [128,512] accumulation slots spanning all
                psum pools (stage A only; stage B owns the pools then)."""
                t0 = ps_s.tile([P, SQC], F32, tag="s")
                t1 = ps_s.tile([P, SQC], F32, tag="s")
                tcx = ps_c.tile([P, SQC], F32, tag="c")
                o0 = ps_o.tile([P, 512], F32, tag="o")
                o1 = ps_o.tile([P, 512], F32, tag="o")
                return [t0[:, 0:512], t0[:, 512:1024],
                        t1[:, 0:512], t1[:, 512:1024],
                        tcx[:, 0:512], tcx[:, 512:1024],
                        o0[:, :], o1[:, :]]

            def dma_keep(c, sqh):
                nc.sync.dma_start(
                    keep_sb[:, c, sqh * SQC:(sqh + 1) * SQC],
                    keep_t[c * P:(c + 1) * P, sqh * SQC:(sqh + 1) * SQC])

            # ---- stage A ----
            # K projection: stream xk, prefetch xv halves into resident xa
            nc.sync.dma_start(wk_sb, wk_t[:].rearrange("p (ko m) -> p ko m", ko=KO))
            nc.sync.dma_start(bk_sb, bk_t[:])
            # dep on wk DMA: covers the DMA-latency head before K's matmuls
            warm_burst(wk_sb[0:1, 0, 0:64], wk_sb[0:1, 0, 0:256], 6)

            slots = psum_slots()
            for kk in range(KO):
                t = xs.tile([P, S], BF16, tag="xt")
                nc.sync.dma_start(t, xk_t[kk * P:(kk + 1) * P, :])
                for g in range(8):
                    m, n = g // 4, g % 4
                    nc.tensor.matmul(
                        slots[g],
                        wk_sb[:, kk, m * P:(m + 1) * P],
                        t[:, n * 512:(n + 1) * 512],
                        start=(kk == 0), stop=(kk == KO - 1),
                    )
                    if kk == KO - 1:
                        nc.vector.tensor_scalar_add(
                            kT_sb[:, m, n * 512:(n + 1) * 512],
                            slots[g], bk_sb[:, m:m + 1])

            nc.sync.dma_start(wv_sb, wv_t[:].rearrange("p (ko m) -> p ko m", ko=KO))
            nc.sync.dma_start(bv_sb, bv_t[:])
            # xv first halves stream just-in-time into wave0
            for kk in range(KO):
                nc.sync.dma_start(xa_sb[:, kk, 0:SQC],
                                  xv_t[kk * P:(kk + 1) * P, 0:SQC])
            nc.sync.dma_start(wq_sb, wq_t[:].rearrange("p (ko m) -> p ko m", ko=KO))
            nc.sync.dma_start(bq_sb, bq_t[:])

            # V projection from resident xa; second xv half + xq ride along
            for w0 in (0, 8):
                slots = psum_slots()
                for kk in range(KO):
                    if w0 == 0:
                        nc.sync.dma_start(xa_sb[:, kk, SQC:S],
                                          xv_t[kk * P:(kk + 1) * P, SQC:S])
                    for g in range(8):
                        sv = w0 + g
                        nc.tensor.matmul(
                            slots[g][:, 0:DQ],
                            xa_sb[:, kk, sv * P:(sv + 1) * P],
                            wv_sb[:, kk, :],
                            start=(kk == 0), stop=(kk == KO - 1),
                        )
                        if kk == KO - 1:
                            nc.vector.tensor_tensor(
                                v_strided[:, sv, :, 0:64],
                                slots[g][:, 0:DQ].rearrange(
                                    "p (h c) -> p h c", h=NH),
                                bv_sb[:].rearrange("p (h c) -> p h c", h=NH),
                                OP.add,
                            )
                    if w0 == 8:
                        # xq overwrites xa[kk] once wave1 has read it
                        nc.sync.dma_start(xa_sb[:, kk, :],
                                          xq_t[kk * P:(kk + 1) * P, :])

            # Q projection, hm=0 half (heads 0,1) — stage B starts after this
            t0 = ps_s.tile([P, SQC], F32, tag="s")
            t1 = ps_s.tile([P, SQC], F32, tag="s")
            qslots = [t0[:, 0:512], t0[:, 512:1024],
                      t1[:, 0:512], t1[:, 512:1024]]
            for kk in range(KO):
                for n in range(4):
                    nc.tensor.matmul(
                        qslots[n],
                        wq_sb[:, kk, 0:P],
                        xa_sb[:, kk, n * 512:(n + 1) * 512],
                        start=(kk == 0), stop=(kk == KO - 1),
                    )
                    if kk == KO - 1:
                        nc.vector.tensor_scalar_add(
                            qT_sb[:, 0, n * 512:(n + 1) * 512],
                            qslots[n], bq_sb[:, 0:1])

            # mask chunks + wo (ride DMA during early stage B, after xq)
            for c in range(SKN):
                dma_keep(c, 0)
            nc.sync.dma_start(wo_sb, wo_t[:].rearrange("p (mq n) -> p mq n", mq=MQ))
            for c in range(SKN):
                dma_keep(c, 1)

            # Q hm=1 half as filler closures, injected into sweep(h0)
            qm1_state = {}

            def qm1_filler(sub, kk):
                if kk == 0:
                    qm1_state[sub] = [
                        ps_o.tile([P, 512], F32, name=f"qm1_{sub}_{i}", tag="o")
                        for i in range(2)]
                tiles = qm1_state[sub]
                for i in range(2):
                    n = sub * 2 + i
                    nc.tensor.matmul(
                        tiles[i],
                        wq_sb[:, kk, P:2 * P],
                        xa_sb[:, kk, n * 512:(n + 1) * 512],
                        start=(kk == 0), stop=(kk == KO - 1),
                    )
                    if kk == KO - 1:
                        nc.vector.tensor_scalar_add(
                            qT_sb[:, 1, n * 512:(n + 1) * 512],
                            tiles[i], bq_sb[:, 1:2])

            fillers = [lambda sub=sub, kk=kk: qm1_filler(sub, kk)
                       for sub in range(2) for kk in range(KO)]

            # ---- stage B: attention, ScalarE-saturated per-head sweeps ----
            def normalize(cps, h, sq0, halves=1):
                """Free cps with one staging copy; normalize lazily.
                HW quirk: custom-DVE / gpsimd ops only work at base partition
                0, so the den row is shifted to partition 0 via SBUF DMA.
                halves=2 pipelines the chain at 512 granularity (short tail)."""
                hb, hm = (h % 2) * 64, h // 2
                w = SQC // halves
                ctxu = scp.tile([65, SQC], F32, tag="ctxu")
                den0 = scp.tile([1, SQC], F32, tag="den0")
                scl = scp.tile([64, SQC], F32, tag="scl")
                cn = scp.tile([64, SQC], BF16, name="cn", tag="cn") if hb else None
                for i in range(halves):
                    s = slice(i * w, (i + 1) * w)
                    so = slice(sq0 + i * w, sq0 + (i + 1) * w)
                    nc.vector.tensor_copy(ctxu[:, s], cps[0:65, s])
                    nc.sync.dma_start(den0[:, s], ctxu[64:65, s])
                    nc.vector.reciprocal_approx_fast(
                        out=den0[:, s], in_=den0[:, s])
                    nc.gpsimd.partition_broadcast(scl[:, s], den0[0:1, s])
                    # multiply on GpSimd: keeps the sweep-end DVE queue clear
                    if hb == 0:
                        nc.gpsimd.tensor_tensor(
                            ctxT_sb[0:64, hm, so], ctxu[0:64, s], scl[:, s],
                            OP.mult)
                    else:
                        nc.gpsimd.tensor_tensor(
                            cn[:, s], ctxu[0:64, s], scl[:, s], OP.mult)
                        nc.sync.dma_start(ctxT_sb[64:128, hm, so], cn[:, s])

            def oproj_unit(do, n2, sq0, eng="v", ps=None):
                if ps is None:
                    ps = ps_o.tile([P, 512], F32, tag="o")
                for kk in range(MQ):
                    nc.tensor.matmul(
                        ps,
                        wo_sb[:, kk, do * P:(do + 1) * P],
                        ctxT_sb[:, kk, sq0 + n2 * 512:sq0 + (n2 + 1) * 512],
                        start=(kk == 0), stop=(kk == MQ - 1),
                    )
                ot = outp.tile([P, 512], F16, tag="ot")
                if eng == "v":
                    nc.vector.tensor_copy(ot, ps)
                else:
                    nc.scalar.copy(ot, ps)
                nc.sync.dma_start(
                    out_t[do * P:(do + 1) * P,
                          sq0 + n2 * 512:sq0 + (n2 + 1) * 512], ot)

            def scores_for(h, sq0, sk):
                hb, hm = (h % 2) * 64, h // 2
                sps = ps_s.tile([P, SQC], F32, name="sps", tag="s")
                for j in range(2):
                    nc.tensor.matmul(
                        sps[:, j * 512:(j + 1) * 512],
                        kT_sb[hb:hb + 64, hm, sk * P:(sk + 1) * P],
                        qT_sb[hb:hb + 64, hm,
                              sq0 + j * 512:sq0 + (j + 1) * 512],
                        start=True, stop=True,
                    )
                return sps

            def sweep(h, sq0, ounits, fill, final=False, pend=None, nxt=None):
                hb, hm = (h % 2) * 64, h // 2
                cps = ps_c.tile([P, SQC], F32, tag="c")

                def ctx(sk, at):
                    for j in range(2):
                        nc.tensor.matmul(
                            cps[:65, j * 512:(j + 1) * 512],
                            v_sb[:, sk, h * 65:(h + 1) * 65],
                            at[:, j * 512:(j + 1) * 512],
                            start=(sk == 0),
                            stop=(sk == SKN - 1),
                        )

                if not pend:
                    pend = [scores_for(h, sq0, 0), scores_for(h, sq0, 1)]
                nso = []
                ats = []
                for sk in range(SKN):
                    sps = pend.pop(0)
                    at = attnp.tile([P, SQC], BF16, tag="at")
                    nc.scalar.activation(at, sps, AF.Exp, scale=0.125)
                    nc.vector.tensor_tensor(
                        at, at, keep_sb[:, sk, sq0:sq0 + SQC], OP.mult)
                    # PE fillers ride the ramp / idle slack
                    for _ in range(2 if sk < 3 else 1):
                        if fill:
                            fill.pop(0)()
                    if sk + 2 < SKN:
                        pend.append(scores_for(h, sq0, sk + 2))
                    if ounits and sk >= 8 and sk % 2 == 0:
                        oproj_unit(*ounits.pop(0))
                    # next sweep's first scores jump ahead of the final ctx
                    # ops so the exp stream crosses the boundary gap-free
                    if sk == SKN - 1 and nxt:
                        nso.append(scores_for(nxt[0], nxt[1], 0))
                    # ctx trails one step so psum hand-off can't block PE
                    ats.append((sk, at))
                    if len(ats) > 1:
                        ctx(*ats.pop(0))
                if nxt:
                    nso.append(scores_for(nxt[0], nxt[1], 1))
                ctx(*ats.pop(0))
                normalize(cps, h, sq0, halves=2 if final else 1)
                return nso

            ounits = []
            # odd heads first: the tail-gating last sweeps then write
            # their normalized ctx straight into ctxT (no shift DMA)
            order = [(h, sqh * SQC) for sqh in range(NSQ)
                     for h in (1, 3, 0, 2)]
            pend = None
            for idx, (h, sq0) in enumerate(order):
                nxt = order[idx + 1] if idx + 1 < len(order) else None
                pend = sweep(h, sq0, ounits,
                             fillers if idx == 0 else None,
                             final=(idx == len(order) - 1),
                             pend=pend, nxt=nxt)
                if idx % NH == NH - 1:
                    ounits.extend((do, n2, sq0)
                                  for do in range(KO) for n2 in range(2))

            # tail drain: keep the PE warm through the last normalize chain,
            # then rotate units through 6 psum slots (score pool is free now)
            wtail = ps_s.tile([P, SQC], F32, tag="s")
            warm_burst(wmm[0:1, 0:64], wmm[0:1, 0:512], 28, dst=wtail)
            t2 = ps_s.tile([P, SQC], F32, tag="s")
            tail_slots = [wtail[:, 0:512], wtail[:, 512:1024],
                          t2[:, 0:512], t2[:, 512:1024], None, None]
            for i, u in enumerate(ounits):
                oproj_unit(*u, eng="v" if i % 2 else "s",
                           ps=tail_slots[i % 6])

    nc.compile()
    return nc


_NC_CACHE = {}


def _get_nc():
    if "nc" not in _NC_CACHE:
        _NC_CACHE["nc"] = build_nc()
    return _NC_CACHE["nc"]


def _pack_w(wT, ko):
    """[D, M] weight (already transposed) -> [P, ko*M] in SBUF layout."""
    d, m = wT.shape
    return np.ascontiguousarray(
        wT.reshape(ko, P, m).transpose(1, 0, 2).reshape(P, ko * m))


def make_in_maps(query, key, value, mask, Wq, bq, Wk, bk, Wv, bv, Wo, bo):
    """Build the 8 per-core input maps (host-side shard + layout prep)."""
    nb = query.shape[0]
    per_b = []
    for b in range(nb):
        xqT = np.ascontiguousarray(query[b].T).astype(NP_BF16)
        xkT = np.ascontiguousarray(key[b].T).astype(NP_BF16)
        xvT = np.ascontiguousarray(value[b].T).astype(NP_BF16)
        keepT = np.ascontiguousarray((~mask[b, 0]).T).astype(NP_BF16)
        per_b.append((xqT, xkT, xvT, keepT))
    per_g = []
    for g in range(4):
        sl = slice(g * DQ, (g + 1) * DQ)
        per_g.append((
            _pack_w(Wq[sl].T.astype(NP_BF16), KO),
            _pack_w(Wk[sl].T.astype(NP_BF16), KO),
            _pack_w(Wv[sl].T.astype(NP_BF16), KO),
            _pack_w(Wo[:, sl].T.astype(NP_BF16), MQ),
            np.ascontiguousarray(bq[sl].reshape(DQ // P, P).T).astype(np.float32),
            np.ascontiguousarray(bk[sl].reshape(DQ // P, P).T).astype(np.float32),
            np.ascontiguousarray(np.broadcast_to(bv[sl], (P, DQ))).astype(np.float32),
        ))
    in_maps = []
    for c in range(NCORES):
        b, g = c // 4, c % 4
        xqT, xkT, xvT, keepT = per_b[b % nb]
        wqT, wkT, wvT, woT, bq2, bk2, bvr = per_g[g]
        in_maps.append({
            "xq": xqT, "xk": xkT, "xv": xvT,
            "wq": wqT, "wk": wkT, "wv": wvT, "wo": woT,
            "bq": bq2, "bk": bk2, "bv": bvr,
            "keep": keepT,
        })
    return in_maps


def gather_output(results, bo, nb=B, s=S, d=D):
    out = np.empty((nb, s, d), np.float32)
    for b in range(nb):
        acc = results[4 * b]["out"].astype(np.float32)
        for g in range(1, 4):
            acc += results[4 * b + g]["out"].astype(np.float32)
        out[b] = acc.T
    out += bo.astype(np.float32)
    return out


def run_on_cores(in_maps, trace=False, **kw):
    nc = _get_nc()
    return run_bass_kernel_spmd(nc, in_maps, list(range(NCORES)), trace=trace, **kw)


def kernel(query, key, value, mask, Wq, bq, Wk, bk, Wv, bv, Wo, bo):
    in_maps = make_in_maps(query, key, value, mask,
                           Wq, bq, Wk, bk, Wv, bv, Wo, bo)
    res = run_on_cores(in_maps, trace=False)
    return gather_output(res.results, bo)


# revision 30
# speedup vs baseline: 1.3119x; 1.3119x over previous
"""Multi-head attention Bass kernel for Trainium2, sharded over 8 NeuronCores.

Sharding: core c handles batch b = c//4 and head-group g = c%4 (4 of 16 heads,
i.e. a 256-wide slice of the QKV projection output).  Each core computes its
heads' attention and a partial output projection (contribution of its 256
ctx columns to the full [S, D] output).  The host sums the 4 partials per
batch (fp32) and adds the output bias.

Device-side design (v4 — ScalarE-saturated pipeline, HAM kept warm):
  - activations shipped pre-transposed: xT = x.T  [D, S]; weights shipped
    pre-packed in the exact SBUF layout (contiguous DMA).
  - scores computed transposed (scoresT[sk, sq]) so attention weights leave
    softmax with sk on partitions — the contraction layout attn@V needs.
  - softmax denominator comes free from a ones-column appended to V
    (ctx psum row 64 = sum_sk attn);  no max-subtraction (scores bounded).
  - stage B processes one head at a time, sweeping sk chunks with a
    double-buffered score psum: the PE issues scores(sk+2) the moment
    exp(sk) frees a buffer, so the Exp stream on ScalarE (the critical
    engine: ~1 elem/cycle/lane) never waits.  ctx matmuls trail one step
    so psum hand-offs can't head-of-line-block the PE queue.
  - HAM (PE clock gate) management: dummy warm-up matmuls cover the DMA
    latency head; the Q projection is split — its hm=1 half is injected
    as filler matmuls into the first sweep so the PE has no idle window
    at the stage A->B transition and stays at 2.4 GHz.
  - normalize frees ctx-psum with one staging copy and runs the
    recip/broadcast/divide lazily; even heads write straight into ctxT.
  - output projection is emitted one unit per sweep step into the PE's
    idle slack during the NEXT sq block's sweeps; final block drains in a
    short tail with psum->SBUF copies split across Vector/Scalar.
  - psum budget (16KB/part): score 2x[128,1024]f32 + ctx 1x[128,1024]f32
    + oproj 2x[128,512]f32 = 8 banks exactly.  Stage A reuses all four
    pools as 8 independent 512-wide accumulation slots, kk-outer.
"""

import numpy as np
import ml_dtypes

import concourse.bass as bass
import concourse.mybir as mybir
import concourse.tile as tile
from concourse import bacc, library_config
from concourse.bass_utils import run_bass_kernel_spmd

# Problem shapes (hardcoded per contest rules).
B, S, D, H, DH = 2, 2048, 1024, 16, 64
NCORES = 8
NH = 4            # heads per core
DQ = NH * DH      # 256: per-core q/k/v width
P = 128

F32 = mybir.dt.float32
F16 = mybir.dt.float16
BF16 = mybir.dt.bfloat16
NP_BF16 = ml_dtypes.bfloat16

SQC = 1024        # sq block per sweep
NSQ = S // SQC    # 2
SKN = S // P      # 16 sk chunks
KO = D // P       # 8 contraction chunks for projections
MQ = DQ // P      # 2


def build_nc():
    """Build the per-core Bass program (same NEFF on all 8 cores)."""
    nc = bacc.Bacc("TRN2", debug=False)

    xq_t = nc.declare_dram_parameter("xq", [D, S], BF16, isOutput=False)
    xk_t = nc.declare_dram_parameter("xk", [D, S], BF16, isOutput=False)
    xv_t = nc.declare_dram_parameter("xv", [D, S], BF16, isOutput=False)
    wq_t = nc.declare_dram_parameter("wq", [P, KO * DQ], BF16, isOutput=False)
    wk_t = nc.declare_dram_parameter("wk", [P, KO * DQ], BF16, isOutput=False)
    wv_t = nc.declare_dram_parameter("wv", [P, KO * DQ], BF16, isOutput=False)
    wo_t = nc.declare_dram_parameter("wo", [P, MQ * D], BF16, isOutput=False)
    bq_t = nc.declare_dram_parameter("bq", [P, MQ], F32, isOutput=False)
    bk_t = nc.declare_dram_parameter("bk", [P, MQ], F32, isOutput=False)
    bv_t = nc.declare_dram_parameter("bv", [P, DQ], F32, isOutput=False)
    keep_t = nc.declare_dram_parameter("keep", [S, S], BF16, isOutput=False)
    out_t = nc.declare_dram_parameter("out", [D, S], F16, isOutput=True)

    AF = mybir.ActivationFunctionType
    OP = mybir.AluOpType

    with tile.TileContext(nc) as tc:
        nc.gpsimd.load_library(library_config.attn)
        with (
            tc.tile_pool(name="const", bufs=1) as const,
            tc.tile_pool(name="xs", bufs=3) as xs,
            tc.tile_pool(name="attn", bufs=9) as attnp,
            tc.tile_pool(name="sc", bufs=2) as scp,
            tc.tile_pool(name="outp", bufs=3) as outp,
            tc.tile_pool(name="ps_s", bufs=2, space="PSUM") as ps_s,
            tc.tile_pool(name="ps_c", bufs=1, space="PSUM") as ps_c,
            tc.tile_pool(name="ps_o", bufs=2, space="PSUM") as ps_o,
        ):
            # ---- persistent SBUF tensors ----
            wq_sb = const.tile([P, KO, DQ], BF16, tag="wq")
            wk_sb = const.tile([P, KO, DQ], BF16, tag="wk")
            wv_sb = const.tile([P, KO, DQ], BF16, tag="wv")
            wo_sb = const.tile([P, MQ, D], BF16, tag="wo")
            bq_sb = const.tile([P, MQ], F32, tag="bq")
            bk_sb = const.tile([P, MQ], F32, tag="bk")
            bv_sb = const.tile([P, DQ], F32, tag="bv")
            qT_sb = const.tile([P, MQ, S], BF16, tag="qT")
            kT_sb = const.tile([P, MQ, S], BF16, tag="kT")
            v_sb = const.tile([P, SKN, NH * 65], BF16, tag="v")
            keep_sb = const.tile([P, SKN, S], BF16, tag="keep")
            ctxT_sb = const.tile([P, MQ, S], BF16, tag="ctxT")
            xa_sb = const.tile([P, KO, S], BF16, tag="xa")  # resident xv->xq
            warm = const.tile([1, 8], F32, tag="warm")
            wmm = const.tile([1, 512], BF16, tag="wmm")

            # preload the exp table set on ScalarE while stage A runs
            nc.vector.memset(warm, 0.0)
            nc.scalar.activation(warm, warm, AF.Exp, scale=1.0)
            nc.vector.memset(wmm, 0.0)

            pswarm = ps_o.tile([P, 512], F32, tag="o")

            def warm_burst(lhs, rhs, n=10, dst=None):
                """Dummy matmuls to keep the PE HAM clock-gate open."""
                w = rhs.shape[-1]
                if dst is None:
                    dst = pswarm
                for _ in range(n):
                    nc.tensor.matmul(dst[0:64, 0:w], lhs,
                                     rhs, start=True, stop=True)

            warm_burst(wmm[0:1, 0:64], wmm[0:1, 0:512], 16)

            # ones column per head in the V tile (softmax denominator trick)
            v_strided = v_sb[:].rearrange("p s (h c) -> p s h c", h=NH)
            nc.vector.memset(v_strided[:, :, :, 64:65], 1.0)

            def psum_slots():
                """8 independent [128,512] accumulation slots spanning all
                psum pools (stage A only; stage B owns the pools then)."""
                t0 = ps_s.tile([P, SQC], F32, tag="s")
                t1 = ps_s.tile([P, SQC], F32, tag="s")
                tcx = ps_c.tile([P, SQC], F32, tag="c")
                o0 = ps_o.tile([P, 512], F32, tag="o")
                o1 = ps_o.tile([P, 512], F32, tag="o")
                return [t0[:, 0:512], t0[:, 512:1024],
                        t1[:, 0:512], t1[:, 512:1024],
                        tcx[:, 0:512], tcx[:, 512:1024],
                        o0[:, :], o1[:, :]]

            def dma_keep(c, sqh):
                nc.sync.dma_start(
                    keep_sb[:, c, sqh * SQC:(sqh + 1) * SQC],
                    keep_t[c * P:(c + 1) * P, sqh * SQC:(sqh + 1) * SQC])

            # ---- stage A ----
            # K projection: stream xk, prefetch xv halves into resident xa
            nc.sync.dma_start(wk_sb, wk_t[:].rearrange("p (ko m) -> p ko m", ko=KO))
            nc.sync.dma_start(bk_sb, bk_t[:])
            # dep on wk DMA: covers the DMA-latency head before K's matmuls
            warm_burst(wk_sb[0:1, 0, 0:64], wk_sb[0:1, 0, 0:256], 6)

            slots = psum_slots()
            for kk in range(KO):
                t = xs.tile([P, S], BF16, tag="xt")
                nc.sync.dma_start(t, xk_t[kk * P:(kk + 1) * P, :])
                for g in range(8):
                    m, n = g // 4, g % 4
                    nc.tensor.matmul(
                        slots[g],
                        wk_sb[:, kk, m * P:(m + 1) * P],
                        t[:, n * 512:(n + 1) * 512],
                        start=(kk == 0), stop=(kk == KO - 1),
                    )
                    if kk == KO - 1:
                        nc.vector.tensor_scalar_add(
                            kT_sb[:, m, n * 512:(n + 1) * 512],
                            slots[g], bk_sb[:, m:m + 1])

            nc.sync.dma_start(wv_sb, wv_t[:].rearrange("p (ko m) -> p ko m", ko=KO))
            nc.sync.dma_start(bv_sb, bv_t[:])
            # xv first halves stream just-in-time into wave0
            for kk in range(KO):
                nc.sync.dma_start(xa_sb[:, kk, 0:SQC],
                                  xv_t[kk * P:(kk + 1) * P, 0:SQC])
            nc.sync.dma_start(wq_sb, wq_t[:].rearrange("p (ko m) -> p ko m", ko=KO))
            nc.sync.dma_start(bq_sb, bq_t[:])

            # V projection from resident xa; second xv half + xq ride along
            for w0 in (0, 8):
                slots = psum_slots()
                for kk in range(KO):
                    if w0 == 0:
                        nc.sync.dma_start(xa_sb[:, kk, SQC:S],
                                          xv_t[kk * P:(kk + 1) * P, SQC:S])
                    for g in range(8):
                        sv = w0 + g
                        nc.tensor.matmul(
                            slots[g][:, 0:DQ],
                            xa_sb[:, kk, sv * P:(sv + 1) * P],
                            wv_sb[:, kk, :],
                            start=(kk == 0), stop=(kk == KO - 1),
                        )
                        if kk == KO - 1:
                            nc.vector.tensor_tensor(
                                v_strided[:, sv, :, 0:64],
                                slots[g][:, 0:DQ].rearrange(
                                    "p (h c) -> p h c", h=NH),
                                bv_sb[:].rearrange("p (h c) -> p h c", h=NH),
                                OP.add,
                            )
                    if w0 == 8:
                        # xq overwrites xa[kk] once wave1 has read it
                        nc.sync.dma_start(xa_sb[:, kk, :],
                                          xq_t[kk * P:(kk + 1) * P, :])

            # Q projection, hm=0 half (heads 0,1) — stage B starts after this
            t0 = ps_s.tile([P, SQC], F32, tag="s")
            t1 = ps_s.tile([P, SQC], F32, tag="s")
            qslots = [t0[:, 0:512], t0[:, 512:1024],
                      t1[:, 0:512], t1[:, 512:1024]]
            for kk in range(KO):
                for n in range(4):
                    nc.tensor.matmul(
                        qslots[n],
                        wq_sb[:, kk, 0:P],
                        xa_sb[:, kk, n * 512:(n + 1) * 512],
                        start=(kk == 0), stop=(kk == KO - 1),
                    )
                    if kk == KO - 1:
                        nc.vector.tensor_scalar_add(
                            qT_sb[:, 0, n * 512:(n + 1) * 512],
                            qslots[n], bq_sb[:, 0:1])

            # mask chunks + wo (ride DMA during early stage B, after xq)
            for c in range(SKN):
                dma_keep(c, 0)
            nc.sync.dma_start(wo_sb, wo_t[:].rearrange("p (mq n) -> p mq n", mq=MQ))
            for c in range(SKN):
                dma_keep(c, 1)

            # Q hm=1 half as filler closures, injected into sweep(h0)
            qm1_state = {}

            def qm1_filler(sub, kk):
                if kk == 0:
                    qm1_state[sub] = [
                        ps_o.tile([P, 512], F32, name=f"qm1_{sub}_{i}", tag="o")
                        for i in range(2)]
                tiles = qm1_state[sub]
                for i in range(2):
                    n = sub * 2 + i
                    nc.tensor.matmul(
                        tiles[i],
                        wq_sb[:, kk, P:2 * P],
                        xa_sb[:, kk, n * 512:(n + 1) * 512],
                        start=(kk == 0), stop=(kk == KO - 1),
                    )
                    if kk == KO - 1:
                        nc.vector.tensor_scalar_add(
                            qT_sb[:, 1, n * 512:(n + 1) * 512],
                            tiles[i], bq_sb[:, 1:2])

            fillers = [lambda sub=sub, kk=kk: qm1_filler(sub, kk)
                       for sub in range(2) for kk in range(KO)]

            # ---- stage B: attention, ScalarE-saturated per-head sweeps ----
            def normalize(cps, h, sq0, halves=1):
                """Free cps with one staging copy; normalize lazily.
                HW quirk: custom-DVE / gpsimd ops only work at base partition
                0, so the den row is shifted to partition 0 via SBUF DMA.
                halves=2 pipelines the chain at 512 granularity (short tail)."""
                hb, hm = (h % 2) * 64, h // 2
                w = SQC // halves
                ctxu = scp.tile([65, SQC], F32, tag="ctxu")
                den0 = scp.tile([1, SQC], F32, tag="den0")
                scl = scp.tile([64, SQC], F32, tag="scl")
                cn = scp.tile([64, SQC], BF16, name="cn", tag="cn") if hb else None
                for i in range(halves):
                    s = slice(i * w, (i + 1) * w)
                    so = slice(sq0 + i * w, sq0 + (i + 1) * w)
                    nc.vector.tensor_copy(ctxu[:, s], cps[0:65, s])
                    nc.sync.dma_start(den0[:, s], ctxu[64:65, s])
                    nc.vector.reciprocal_approx_fast(
                        out=den0[:, s], in_=den0[:, s])
                    nc.gpsimd.partition_broadcast(scl[:, s], den0[0:1, s])
                    if hb == 0:
                        nc.vector.tensor_tensor(
                            ctxT_sb[0:64, hm, so], ctxu[0:64, s], scl[:, s],
                            OP.mult)
                    else:
                        nc.vector.tensor_tensor(
                            cn[:, s], ctxu[0:64, s], scl[:, s], OP.mult)
                        nc.sync.dma_start(ctxT_sb[64:128, hm, so], cn[:, s])

            def oproj_unit(do, n2, sq0, eng="v", ps=None):
                if ps is None:
                    ps = ps_o.tile([P, 512], F32, tag="o")
                for kk in range(MQ):
                    nc.tensor.matmul(
                        ps,
                        wo_sb[:, kk, do * P:(do + 1) * P],
                        ctxT_sb[:, kk, sq0 + n2 * 512:sq0 + (n2 + 1) * 512],
                        start=(kk == 0), stop=(kk == MQ - 1),
                    )
                ot = outp.tile([P, 512], F16, tag="ot")
                if eng == "v":
                    nc.vector.tensor_copy(ot, ps)
                else:
                    nc.scalar.copy(ot, ps)
                nc.sync.dma_start(
                    out_t[do * P:(do + 1) * P,
                          sq0 + n2 * 512:sq0 + (n2 + 1) * 512], ot)

            def scores_for(h, sq0, sk):
                hb, hm = (h % 2) * 64, h // 2
                sps = ps_s.tile([P, SQC], F32, name="sps", tag="s")
                for j in range(2):
                    nc.tensor.matmul(
                        sps[:, j * 512:(j + 1) * 512],
                        kT_sb[hb:hb + 64, hm, sk * P:(sk + 1) * P],
                        qT_sb[hb:hb + 64, hm,
                              sq0 + j * 512:sq0 + (j + 1) * 512],
                        start=True, stop=True,
                    )
                return sps

            def sweep(h, sq0, ounits, fill, final=False, pend=None, nxt=None):
                hb, hm = (h % 2) * 64, h // 2
                cps = ps_c.tile([P, SQC], F32, tag="c")

                def ctx(sk, at):
                    for j in range(2):
                        nc.tensor.matmul(
                            cps[:65, j * 512:(j + 1) * 512],
                            v_sb[:, sk, h * 65:(h + 1) * 65],
                            at[:, j * 512:(j + 1) * 512],
                            start=(sk == 0),
                            stop=(sk == SKN - 1),
                        )

                if not pend:
                    pend = [scores_for(h, sq0, 0), scores_for(h, sq0, 1)]
                nso = []
                ats = []
                for sk in range(SKN):
                    sps = pend.pop(0)
                    at = attnp.tile([P, SQC], BF16, tag="at")
                    nc.scalar.activation(at, sps, AF.Exp, scale=0.125)
                    nc.vector.tensor_tensor(
                        at, at, keep_sb[:, sk, sq0:sq0 + SQC], OP.mult)
                    # PE fillers ride the ramp / idle slack
                    for _ in range(2 if sk < 3 else 1):
                        if fill:
                            fill.pop(0)()
                    if sk + 2 < SKN:
                        pend.append(scores_for(h, sq0, sk + 2))
                    if ounits and sk >= 8 and sk % 2 == 0:
                        oproj_unit(*ounits.pop(0))
                    # next sweep's first scores jump ahead of the final ctx
                    # ops so the exp stream crosses the boundary gap-free
                    if sk == SKN - 1 and nxt:
                        nso.append(scores_for(nxt[0], nxt[1], 0))
                    # ctx trails one step so psum hand-off can't block PE
                    ats.append((sk, at))
                    if len(ats) > 1:
                        ctx(*ats.pop(0))
                if nxt:
                    nso.append(scores_for(nxt[0], nxt[1], 1))
                ctx(*ats.pop(0))
                normalize(cps, h, sq0, halves=2 if final else 1)
                return nso

            ounits = []
            # odd heads first: the tail-gating last sweeps then write
            # their normalized ctx straight into ctxT (no shift DMA)
            order = [(h, sqh * SQC) for sqh in range(NSQ)
                     for h in (1, 3, 0, 2)]
            pend = None
            for idx, (h, sq0) in enumerate(order):
                nxt = order[idx + 1] if idx + 1 < len(order) else None
                pend = sweep(h, sq0, ounits,
                             fillers if idx == 0 else None,
                             final=(idx == len(order) - 1),
                             pend=pend, nxt=nxt)
                if idx % NH == NH - 1:
                    ounits.extend((do, n2, sq0)
                                  for do in range(KO) for n2 in range(2))

            # tail drain: keep the PE warm through the last normalize chain,
            # then rotate units through 6 psum slots (score pool is free now)
            wtail = ps_s.tile([P, SQC], F32, tag="s")
            warm_burst(wmm[0:1, 0:64], wmm[0:1, 0:512], 28, dst=wtail)
            t2 = ps_s.tile([P, SQC], F32, tag="s")
            tail_slots = [wtail[:, 0:512], wtail[:, 512:1024],
                          t2[:, 0:512], t2[:, 512:1024], None, None]
            for i, u in enumerate(ounits):
                oproj_unit(*u, eng="v" if i % 2 else "s",
                           ps=tail_slots[i % 6])

    nc.compile()
    return nc


_NC_CACHE = {}


def _get_nc():
    if "nc" not in _NC_CACHE:
        _NC_CACHE["nc"] = build_nc()
    return _NC_CACHE["nc"]


def _pack_w(wT, ko):
    """[D, M] weight (already transposed) -> [P, ko*M] in SBUF layout."""
    d, m = wT.shape
    return np.ascontiguousarray(
        wT.reshape(ko, P, m).transpose(1, 0, 2).reshape(P, ko * m))


def make_in_maps(query, key, value, mask, Wq, bq, Wk, bk, Wv, bv, Wo, bo):
    """Build the 8 per-core input maps (host-side shard + layout prep)."""
    nb = query.shape[0]
    per_b = []
    for b in range(nb):
        xqT = np.ascontiguousarray(query[b].T).astype(NP_BF16)
        xkT = np.ascontiguousarray(key[b].T).astype(NP_BF16)
        xvT = np.ascontiguousarray(value[b].T).astype(NP_BF16)
        keepT = np.ascontiguousarray((~mask[b, 0]).T).astype(NP_BF16)
        per_b.append((xqT, xkT, xvT, keepT))
    per_g = []
    for g in range(4):
        sl = slice(g * DQ, (g + 1) * DQ)
        per_g.append((
            _pack_w(Wq[sl].T.astype(NP_BF16), KO),
            _pack_w(Wk[sl].T.astype(NP_BF16), KO),
            _pack_w(Wv[sl].T.astype(NP_BF16), KO),
            _pack_w(Wo[:, sl].T.astype(NP_BF16), MQ),
            np.ascontiguousarray(bq[sl].reshape(DQ // P, P).T).astype(np.float32),
            np.ascontiguousarray(bk[sl].reshape(DQ // P, P).T).astype(np.float32),
            np.ascontiguousarray(np.broadcast_to(bv[sl], (P, DQ))).astype(np.float32),
        ))
    in_maps = []
    for c in range(NCORES):
        b, g = c // 4, c % 4
        xqT, xkT, xvT, keepT = per_b[b % nb]
        wqT, wkT, wvT, woT, bq2, bk2, bvr = per_g[g]
        in_maps.append({
            "xq": xqT, "xk": xkT, "xv": xvT,
            "wq": wqT, "wk": wkT, "wv": wvT, "wo": woT,
            "bq": bq2, "bk": bk2, "bv": bvr,
            "keep": keepT,
        })
    return in_maps


def gather_output(results, bo, nb=B, s=S, d=D):
    out = np.empty((nb, s, d), np.float32)
    for b in range(nb):
        acc = results[4 * b]["out"].astype(np.float32)
        for g in range(1, 4):
            acc += results[4 * b + g]["out"].astype(np.float32)
        out[b] = acc.T
    out += bo.astype(np.float32)
    return out


def run_on_cores(in_maps, trace=False, **kw):
    nc = _get_nc()
    return run_bass_kernel_spmd(nc, in_maps, list(range(NCORES)), trace=trace, **kw)


def kernel(query, key, value, mask, Wq, bq, Wk, bk, Wv, bv, Wo, bo):
    in_maps = make_in_maps(query, key, value, mask,
                           Wq, bq, Wk, bk, Wv, bv, Wo, bo)
    res = run_on_cores(in_maps, trace=False)
    return gather_output(res.results, bo)


# revision 39
# speedup vs baseline: 1.3533x; 1.0316x over previous
"""Multi-head attention Bass kernel for Trainium2, sharded over 8 NeuronCores.

Sharding: core c handles batch b = c//4 and head-group g = c%4 (4 of 16 heads,
i.e. a 256-wide slice of the QKV projection output).  Each core computes its
heads' attention and a partial output projection (contribution of its 256
ctx columns to the full [S, D] output).  The host sums the 4 partials per
batch (fp32) and adds the output bias.

Device-side design (v4 — ScalarE-saturated pipeline, HAM kept warm):
  - activations shipped pre-transposed: xT = x.T  [D, S]; weights shipped
    pre-packed in the exact SBUF layout (contiguous DMA).
  - scores computed transposed (scoresT[sk, sq]) so attention weights leave
    softmax with sk on partitions — the contraction layout attn@V needs.
  - softmax denominator comes free from a ones-column appended to V
    (ctx psum row 64 = sum_sk attn);  no max-subtraction (scores bounded).
  - stage B processes one head at a time, sweeping sk chunks with a
    double-buffered score psum: the PE issues scores(sk+2) the moment
    exp(sk) frees a buffer, so the Exp stream on ScalarE (the critical
    engine: ~1 elem/cycle/lane) never waits.  ctx matmuls trail one step
    so psum hand-offs can't head-of-line-block the PE queue.
  - HAM (PE clock gate) management: dummy warm-up matmuls cover the DMA
    latency head; the Q projection is split — its hm=1 half is injected
    as filler matmuls into the first sweep so the PE has no idle window
    at the stage A->B transition and stays at 2.4 GHz.
  - normalize frees ctx-psum with one staging copy and runs the
    recip/broadcast/divide lazily; even heads write straight into ctxT.
  - output projection is emitted one unit per sweep step into the PE's
    idle slack during the NEXT sq block's sweeps; final block drains in a
    short tail with psum->SBUF copies split across Vector/Scalar.
  - psum budget (16KB/part): score 2x[128,1024]f32 + ctx 1x[128,1024]f32
    + oproj 2x[128,512]f32 = 8 banks exactly.  Stage A reuses all four
    pools as 8 independent 512-wide accumulation slots, kk-outer.
"""

import numpy as np
import ml_dtypes

import concourse.bass as bass
import concourse.mybir as mybir
import concourse.tile as tile
from concourse import bacc, library_config
from concourse.bass_utils import run_bass_kernel_spmd

# Problem shapes (hardcoded per contest rules).
B, S, D, H, DH = 2, 2048, 1024, 16, 64
NCORES = 8
NH = 4            # heads per core
DQ = NH * DH      # 256: per-core q/k/v width
P = 128

F32 = mybir.dt.float32
F16 = mybir.dt.float16
BF16 = mybir.dt.bfloat16
NP_BF16 = ml_dtypes.bfloat16

SQC = 1024        # sq block per sweep
NSQ = S // SQC    # 2
SKN = S // P      # 16 sk chunks
KO = D // P       # 8 contraction chunks for projections
MQ = DQ // P      # 2


def build_nc():
    """Build the per-core Bass program (same NEFF on all 8 cores)."""
    nc = bacc.Bacc("TRN2", debug=False)

    xq_t = nc.declare_dram_parameter("xq", [D, S], BF16, isOutput=False)
    xk_t = nc.declare_dram_parameter("xk", [D, S], BF16, isOutput=False)
    xv_t = nc.declare_dram_parameter("xv", [D, S], BF16, isOutput=False)
    wq_t = nc.declare_dram_parameter("wq", [P, KO * DQ], BF16, isOutput=False)
    wk_t = nc.declare_dram_parameter("wk", [P, KO * DQ], BF16, isOutput=False)
    wv_t = nc.declare_dram_parameter("wv", [P, KO * DQ], BF16, isOutput=False)
    wo_t = nc.declare_dram_parameter("wo", [P, MQ * D], BF16, isOutput=False)
    bq_t = nc.declare_dram_parameter("bq", [P, MQ], F32, isOutput=False)
    bk_t = nc.declare_dram_parameter("bk", [P, MQ], F32, isOutput=False)
    bv_t = nc.declare_dram_parameter("bv", [P, DQ], F32, isOutput=False)
    keep_t = nc.declare_dram_parameter("keep", [S, S], BF16, isOutput=False)
    out_t = nc.declare_dram_parameter("out", [D, S], F16, isOutput=True)

    AF = mybir.ActivationFunctionType
    OP = mybir.AluOpType

    with tile.TileContext(nc) as tc:
        nc.gpsimd.load_library(library_config.attn)
        with (
            tc.tile_pool(name="const", bufs=1) as const,
            tc.tile_pool(name="xs", bufs=3) as xs,
            tc.tile_pool(name="attn", bufs=9) as attnp,
            tc.tile_pool(name="sc", bufs=2) as scp,
            tc.tile_pool(name="outp", bufs=3) as outp,
            tc.tile_pool(name="ps_s", bufs=2, space="PSUM") as ps_s,
            tc.tile_pool(name="ps_c", bufs=1, space="PSUM") as ps_c,
            tc.tile_pool(name="ps_o", bufs=2, space="PSUM") as ps_o,
        ):
            # ---- persistent SBUF tensors ----
            wq_sb = const.tile([P, KO, DQ], BF16, tag="wq")
            wk_sb = const.tile([P, KO, DQ], BF16, tag="wk")
            wv_sb = const.tile([P, KO, DQ], BF16, tag="wv")
            wo_sb = const.tile([P, MQ, D], BF16, tag="wo")
            bq_sb = const.tile([P, MQ], F32, tag="bq")
            bk_sb = const.tile([P, MQ], F32, tag="bk")
            bv_sb = const.tile([P, DQ], F32, tag="bv")
            qT_sb = const.tile([P, MQ, S], BF16, tag="qT")
            kT_sb = const.tile([P, MQ, S], BF16, tag="kT")
            v_sb = const.tile([P, SKN, NH * 65], BF16, tag="v")
            keep_sb = const.tile([P, SKN, S], BF16, tag="keep")
            ctxT_sb = const.tile([P, MQ, S], BF16, tag="ctxT")
            xa_sb = const.tile([P, KO, S], BF16, tag="xa")  # resident xv->xq
            warm = const.tile([1, 8], F32, tag="warm")
            wmm = const.tile([P, 512], BF16, tag="wmm")

            # preload the exp table set on ScalarE while stage A runs
            nc.vector.memset(warm, 0.0)
            nc.scalar.activation(warm, warm, AF.Exp, scale=1.0)
            nc.vector.memset(wmm, 0.0)

            pswarm = ps_o.tile([P, 512], F32, tag="o")

            def warm_burst(lhs, rhs, n=10, dst=None):
                """Dummy matmuls keeping the PE HAM clock-gate open.  Full
                128x128 stationary operand: skinny matmuls don't register
                enough array activity for the HAM to stay at K=8/8."""
                w = rhs.shape[-1]
                if dst is None:
                    dst = pswarm
                for _ in range(n):
                    nc.tensor.matmul(dst[:, 0:w], lhs,
                                     rhs, start=True, stop=True)

            warm_burst(wmm[:, 0:128], wmm[:, 0:512], 16)

            # ones column per head in the V tile (softmax denominator trick)
            v_strided = v_sb[:].rearrange("p s (h c) -> p s h c", h=NH)
            nc.vector.memset(v_strided[:, :, :, 64:65], 1.0)

            def psum_slots():
                """8 independent [128,512] accumulation slots spanning all
                psum pools (stage A only; stage B owns the pools then)."""
                t0 = ps_s.tile([P, SQC], F32, tag="s")
                t1 = ps_s.tile([P, SQC], F32, tag="s")
                tcx = ps_c.tile([P, SQC], F32, tag="c")
                o0 = ps_o.tile([P, 512], F32, tag="o")
                o1 = ps_o.tile([P, 512], F32, tag="o")
                return [t0[:, 0:512], t0[:, 512:1024],
                        t1[:, 0:512], t1[:, 512:1024],
                        tcx[:, 0:512], tcx[:, 512:1024],
                        o0[:, :], o1[:, :]]

            def dma_keep(c, sqh):
                nc.sync.dma_start(
                    keep_sb[:, c, sqh * SQC:(sqh + 1) * SQC],
                    keep_t[c * P:(c + 1) * P, sqh * SQC:(sqh + 1) * SQC])

            # ---- stage A ----
            # K projection: stream xk, prefetch xv halves into resident xa
            nc.sync.dma_start(wk_sb, wk_t[:].rearrange("p (ko m) -> p ko m", ko=KO))
            nc.sync.dma_start(bk_sb, bk_t[:])
            # dep on wk DMA: covers the DMA-latency head before K's matmuls
            warm_burst(wk_sb[:, 0, 0:128], wmm[:, 0:512], 6)

            slots = psum_slots()
            for kk in range(KO):
                t = xs.tile([P, S], BF16, tag="xt")
                nc.sync.dma_start(t, xk_t[kk * P:(kk + 1) * P, :])
                for g in range(8):
                    m, n = g // 4, g % 4
                    nc.tensor.matmul(
                        slots[g],
                        wk_sb[:, kk, m * P:(m + 1) * P],
                        t[:, n * 512:(n + 1) * 512],
                        start=(kk == 0), stop=(kk == KO - 1),
                    )
                    if kk == KO - 1:
                        nc.vector.tensor_scalar_add(
                            kT_sb[:, m, n * 512:(n + 1) * 512],
                            slots[g], bk_sb[:, m:m + 1])

            nc.sync.dma_start(wv_sb, wv_t[:].rearrange("p (ko m) -> p ko m", ko=KO))
            nc.sync.dma_start(bv_sb, bv_t[:])
            # xv first halves stream just-in-time into wave0
            for kk in range(KO):
                nc.sync.dma_start(xa_sb[:, kk, 0:SQC],
                                  xv_t[kk * P:(kk + 1) * P, 0:SQC])
            nc.sync.dma_start(wq_sb, wq_t[:].rearrange("p (ko m) -> p ko m", ko=KO))
            nc.sync.dma_start(bq_sb, bq_t[:])

            # V projection from resident xa; second xv half + xq ride along
            for w0 in (0, 8):
                slots = psum_slots()
                for kk in range(KO):
                    if w0 == 0:
                        nc.sync.dma_start(xa_sb[:, kk, SQC:S],
                                          xv_t[kk * P:(kk + 1) * P, SQC:S])
                    for g in range(8):
                        sv = w0 + g
                        nc.tensor.matmul(
                            slots[g][:, 0:DQ],
                            xa_sb[:, kk, sv * P:(sv + 1) * P],
                            wv_sb[:, kk, :],
                            start=(kk == 0), stop=(kk == KO - 1),
                        )
                        if kk == KO - 1:
                            nc.vector.tensor_tensor(
                                v_strided[:, sv, :, 0:64],
                                slots[g][:, 0:DQ].rearrange(
                                    "p (h c) -> p h c", h=NH),
                                bv_sb[:].rearrange("p (h c) -> p h c", h=NH),
                                OP.add,
                            )
                    if w0 == 8:
                        # xq overwrites xa[kk] once wave1 has read it
                        nc.sync.dma_start(xa_sb[:, kk, :],
                                          xq_t[kk * P:(kk + 1) * P, :])

            # Q projection, hm=0 half (heads 0,1) — stage B starts after this
            t0 = ps_s.tile([P, SQC], F32, tag="s")
            t1 = ps_s.tile([P, SQC], F32, tag="s")
            qslots = [t0[:, 0:512], t0[:, 512:1024],
                      t1[:, 0:512], t1[:, 512:1024]]
            for kk in range(KO):
                for n in range(4):
                    nc.tensor.matmul(
                        qslots[n],
                        wq_sb[:, kk, 0:P],
                        xa_sb[:, kk, n * 512:(n + 1) * 512],
                        start=(kk == 0), stop=(kk == KO - 1),
                    )
                    if kk == KO - 1:
                        nc.vector.tensor_scalar_add(
                            qT_sb[:, 0, n * 512:(n + 1) * 512],
                            qslots[n], bq_sb[:, 0:1])

            # mask chunks + wo (ride DMA during early stage B, after xq)
            for c in range(SKN):
                dma_keep(c, 0)
            nc.sync.dma_start(wo_sb, wo_t[:].rearrange("p (mq n) -> p mq n", mq=MQ))
            for c in range(SKN):
                dma_keep(c, 1)

            # Q hm=1 half as filler closures, injected into sweep(h0)
            qm1_state = {}

            def qm1_filler(sub, kk):
                if kk == 0:
                    qm1_state[sub] = [
                        ps_o.tile([P, 512], F32, name=f"qm1_{sub}_{i}", tag="o")
                        for i in range(2)]
                tiles = qm1_state[sub]
                for i in range(2):
                    n = sub * 2 + i
                    nc.tensor.matmul(
                        tiles[i],
                        wq_sb[:, kk, P:2 * P],
                        xa_sb[:, kk, n * 512:(n + 1) * 512],
                        start=(kk == 0), stop=(kk == KO - 1),
                    )
                    if kk == KO - 1:
                        nc.vector.tensor_scalar_add(
                            qT_sb[:, 1, n * 512:(n + 1) * 512],
                            tiles[i], bq_sb[:, 1:2])

            fillers = [lambda sub=sub, kk=kk: qm1_filler(sub, kk)
                       for sub in range(2) for kk in range(KO)]

            # ---- stage B: attention, ScalarE-saturated per-head sweeps ----
            def normalize(cps, h, sq0, halves=1):
                """Free cps with one staging copy; normalize lazily.
                HW quirk: custom-DVE / gpsimd ops only work at base partition
                0, so the den row is shifted to partition 0 via SBUF DMA.
                halves=2 pipelines the chain at 512 granularity (short tail)."""
                hb, hm = (h % 2) * 64, h // 2
                w = SQC // halves
                ctxu = scp.tile([65, SQC], F32, tag="ctxu")
                den0 = scp.tile([1, SQC], F32, tag="den0")
                scl = scp.tile([64, SQC], F32, tag="scl")
                cn = scp.tile([64, SQC], BF16, name="cn", tag="cn") if hb else None
                for i in range(halves):
                    s = slice(i * w, (i + 1) * w)
                    so = slice(sq0 + i * w, sq0 + (i + 1) * w)
                    if halves > 1:
                        # tail fast-path: den row first so the recip chain
                        # doesn't wait behind the big staging copy
                        nc.vector.tensor_copy(ctxu[64:65, s], cps[64:65, s])
                        nc.sync.dma_start(den0[:, s], ctxu[64:65, s])
                        nc.vector.tensor_copy(ctxu[0:64, s], cps[0:64, s])
                    else:
                        nc.vector.tensor_copy(ctxu[:, s], cps[0:65, s])
                        nc.sync.dma_start(den0[:, s], ctxu[64:65, s])
                    nc.vector.reciprocal_approx_fast(
                        out=den0[:, s], in_=den0[:, s])
                    nc.gpsimd.partition_broadcast(scl[:, s], den0[0:1, s])
                    if hb == 0:
                        nc.vector.tensor_tensor(
                            ctxT_sb[0:64, hm, so], ctxu[0:64, s], scl[:, s],
                            OP.mult)
                    else:
                        nc.vector.tensor_tensor(
                            cn[:, s], ctxu[0:64, s], scl[:, s], OP.mult)
                        nc.sync.dma_start(ctxT_sb[64:128, hm, so], cn[:, s])

            def oproj_unit(do, n2, sq0, eng="v", ps=None):
                if ps is None:
                    ps = ps_o.tile([P, 512], F32, tag="o")
                for kk in range(MQ):
                    nc.tensor.matmul(
                        ps,
                        wo_sb[:, kk, do * P:(do + 1) * P],
                        ctxT_sb[:, kk, sq0 + n2 * 512:sq0 + (n2 + 1) * 512],
                        start=(kk == 0), stop=(kk == MQ - 1),
                    )
                ot = outp.tile([P, 512], F16, tag="ot")
                if eng == "v":
                    nc.vector.tensor_copy(ot, ps)
                else:
                    nc.scalar.copy(ot, ps)
                nc.sync.dma_start(
                    out_t[do * P:(do + 1) * P,
                          sq0 + n2 * 512:sq0 + (n2 + 1) * 512], ot)

            def scores_for(h, sq0, sk):
                hb, hm = (h % 2) * 64, h // 2
                sps = ps_s.tile([P, SQC], F32, name="sps", tag="s")
                for j in range(2):
                    nc.tensor.matmul(
                        sps[:, j * 512:(j + 1) * 512],
                        kT_sb[hb:hb + 64, hm, sk * P:(sk + 1) * P],
                        qT_sb[hb:hb + 64, hm,
                              sq0 + j * 512:sq0 + (j + 1) * 512],
                        start=True, stop=True,
                    )
                return sps

            def sweep(h, sq0, ounits, fill, final=False, pend=None, nxt=None):
                hb, hm = (h % 2) * 64, h // 2
                cps = ps_c.tile([P, SQC], F32, tag="c")

                def ctx(sk, at):
                    for j in range(2):
                        nc.tensor.matmul(
                            cps[:65, j * 512:(j + 1) * 512],
                            v_sb[:, sk, h * 65:(h + 1) * 65],
                            at[:, j * 512:(j + 1) * 512],
                            start=(sk == 0),
                            stop=(sk == SKN - 1),
                        )

                if not pend:
                    pend = [scores_for(h, sq0, 0), scores_for(h, sq0, 1)]
                nso = []
                ats = []
                for sk in range(SKN):
                    sps = pend.pop(0)
                    at = attnp.tile([P, SQC], BF16, tag="at")
                    nc.scalar.activation(at, sps, AF.Exp, scale=0.125)
                    nc.vector.tensor_tensor(
                        at, at, keep_sb[:, sk, sq0:sq0 + SQC], OP.mult)
                    # PE fillers ride the ramp / idle slack
                    for _ in range(2 if sk < 3 else 1):
                        if fill:
                            fill.pop(0)()
                    if sk + 2 < SKN:
                        pend.append(scores_for(h, sq0, sk + 2))
                    if ounits and sk >= 8 and sk % 2 == 0:
                        oproj_unit(*ounits.pop(0))
                    # next sweep's first scores jump ahead of the final ctx
                    # ops so the exp stream crosses the boundary gap-free
                    if sk == SKN - 1 and nxt:
                        nso.append(scores_for(nxt[0], nxt[1], 0))
                    # ctx trails one step so psum hand-off can't block PE
                    ats.append((sk, at))
                    if len(ats) > 1:
                        ctx(*ats.pop(0))
                if nxt:
                    nso.append(scores_for(nxt[0], nxt[1], 1))
                while ats:
                    ctx(*ats.pop(0))
                normalize(cps, h, sq0, halves=2 if final else 1)
                return nso

            ounits = []
            # odd heads first: the tail-gating last sweeps then write
            # their normalized ctx straight into ctxT (no shift DMA)
            order = [(h, sqh * SQC) for sqh in range(NSQ)
                     for h in (1, 3, 0, 2)]
            pend = None
            for idx, (h, sq0) in enumerate(order):
                nxt = order[idx + 1] if idx + 1 < len(order) else None
                pend = sweep(h, sq0, ounits,
                             fillers if idx == 0 else None,
                             final=(idx == len(order) - 1),
                             pend=pend, nxt=nxt)
                if idx % NH == NH - 1:
                    ounits.extend((do, n2, sq0)
                                  for do in range(KO) for n2 in range(2))

            # tail drain: keep the PE warm through the last normalize chain,
            # then rotate units through 6 psum slots (score pool is free now)
            wtail = ps_s.tile([P, SQC], F32, tag="s")
            warm_burst(wq_sb[:, 0, 0:128], qT_sb[:, 0, 0:512], 24, dst=wtail)
            t2 = ps_s.tile([P, SQC], F32, tag="s")
            tail_slots = [wtail[:, 0:512], wtail[:, 512:1024],
                          t2[:, 0:512], t2[:, 512:1024], None, None]
            for i, u in enumerate(ounits):
                oproj_unit(*u, eng="v" if i % 2 else "s",
                           ps=tail_slots[i % 6])

    nc.compile()
    return nc


_NC_CACHE = {}


def _get_nc():
    if "nc" not in _NC_CACHE:
        _NC_CACHE["nc"] = build_nc()
    return _NC_CACHE["nc"]


def _pack_w(wT, ko):
    """[D, M] weight (already transposed) -> [P, ko*M] in SBUF layout."""
    d, m = wT.shape
    return np.ascontiguousarray(
        wT.reshape(ko, P, m).transpose(1, 0, 2).reshape(P, ko * m))


def make_in_maps(query, key, value, mask, Wq, bq, Wk, bk, Wv, bv, Wo, bo):
    """Build the 8 per-core input maps (host-side shard + layout prep)."""
    nb = query.shape[0]
    per_b = []
    for b in range(nb):
        xqT = np.ascontiguousarray(query[b].T).astype(NP_BF16)
        xkT = np.ascontiguousarray(key[b].T).astype(NP_BF16)
        xvT = np.ascontiguousarray(value[b].T).astype(NP_BF16)
        keepT = np.ascontiguousarray((~mask[b, 0]).T).astype(NP_BF16)
        per_b.append((xqT, xkT, xvT, keepT))
    per_g = []
    for g in range(4):
        sl = slice(g * DQ, (g + 1) * DQ)
        per_g.append((
            _pack_w(Wq[sl].T.astype(NP_BF16), KO),
            _pack_w(Wk[sl].T.astype(NP_BF16), KO),
            _pack_w(Wv[sl].T.astype(NP_BF16), KO),
            _pack_w(Wo[:, sl].T.astype(NP_BF16), MQ),
            np.ascontiguousarray(bq[sl].reshape(DQ // P, P).T).astype(np.float32),
            np.ascontiguousarray(bk[sl].reshape(DQ // P, P).T).astype(np.float32),
            np.ascontiguousarray(np.broadcast_to(bv[sl], (P, DQ))).astype(np.float32),
        ))
    in_maps = []
    for c in range(NCORES):
        b, g = c // 4, c % 4
        xqT, xkT, xvT, keepT = per_b[b % nb]
        wqT, wkT, wvT, woT, bq2, bk2, bvr = per_g[g]
        in_maps.append({
            "xq": xqT, "xk": xkT, "xv": xvT,
            "wq": wqT, "wk": wkT, "wv": wvT, "wo": woT,
            "bq": bq2, "bk": bk2, "bv": bvr,
            "keep": keepT,
        })
    return in_maps


def gather_output(results, bo, nb=B, s=S, d=D):
    out = np.empty((nb, s, d), np.float32)
    for b in range(nb):
        acc = results[4 * b]["out"].astype(np.float32)
        for g in range(1, 4):
            acc += results[4 * b + g]["out"].astype(np.float32)
        out[b] = acc.T
    out += bo.astype(np.float32)
    return out


def run_on_cores(in_maps, trace=False, **kw):
    nc = _get_nc()
    return run_bass_kernel_spmd(nc, in_maps, list(range(NCORES)), trace=trace, **kw)


def kernel(query, key, value, mask, Wq, bq, Wk, bk, Wv, bv, Wo, bo):
    in_maps = make_in_maps(query, key, value, mask,
                           Wq, bq, Wk, bk, Wv, bv, Wo, bo)
    res = run_on_cores(in_maps, trace=False)
    return gather_output(res.results, bo)


# revision 52
# speedup vs baseline: 1.3735x; 1.0149x over previous
"""Multi-head attention Bass kernel for Trainium2, sharded over 8 NeuronCores.

Sharding: core c handles batch b = c//4 and head-group g = c%4 (4 of 16 heads,
i.e. a 256-wide slice of the QKV projection output).  Each core computes its
heads' attention and a partial output projection (contribution of its 256
ctx columns to the full [S, D] output).  The host sums the 4 partials per
batch (fp32) and adds the output bias.

Device-side design (v4 — ScalarE-saturated pipeline, HAM kept warm):
  - activations shipped pre-transposed: xT = x.T  [D, S]; weights shipped
    pre-packed in the exact SBUF layout (contiguous DMA).
  - scores computed transposed (scoresT[sk, sq]) so attention weights leave
    softmax with sk on partitions — the contraction layout attn@V needs.
  - softmax denominator comes free from a ones-column appended to V
    (ctx psum row 64 = sum_sk attn);  no max-subtraction (scores bounded).
  - stage B processes one head at a time, sweeping sk chunks with a
    double-buffered score psum: the PE issues scores(sk+2) the moment
    exp(sk) frees a buffer, so the Exp stream on ScalarE (the critical
    engine: ~1 elem/cycle/lane) never waits.  ctx matmuls trail one step
    so psum hand-offs can't head-of-line-block the PE queue.
  - HAM (PE clock gate) management: dummy warm-up matmuls cover the DMA
    latency head; the Q projection is split — its hm=1 half is injected
    as filler matmuls into the first sweep so the PE has no idle window
    at the stage A->B transition and stays at 2.4 GHz.
  - normalize frees ctx-psum with one staging copy and runs the
    recip/broadcast/divide lazily; even heads write straight into ctxT.
  - output projection is emitted one unit per sweep step into the PE's
    idle slack during the NEXT sq block's sweeps; final block drains in a
    short tail with psum->SBUF copies split across Vector/Scalar.
  - psum budget (16KB/part): score 2x[128,1024]f32 + ctx 1x[128,1024]f32
    + oproj 2x[128,512]f32 = 8 banks exactly.  Stage A reuses all four
    pools as 8 independent 512-wide accumulation slots, kk-outer.
"""

import numpy as np
import ml_dtypes

import concourse.bass as bass
import concourse.mybir as mybir
import concourse.tile as tile
from concourse import bacc, library_config
from concourse.bass_utils import run_bass_kernel_spmd

# Problem shapes (hardcoded per contest rules).
B, S, D, H, DH = 2, 2048, 1024, 16, 64
NCORES = 8
NH = 4            # heads per core
DQ = NH * DH      # 256: per-core q/k/v width
P = 128

F32 = mybir.dt.float32
F16 = mybir.dt.float16
BF16 = mybir.dt.bfloat16
NP_BF16 = ml_dtypes.bfloat16

SQC = 1024        # sq block per sweep
NSQ = S // SQC    # 2
SKN = S // P      # 16 sk chunks
KO = D // P       # 8 contraction chunks for projections
MQ = DQ // P      # 2


def build_nc():
    """Build the per-core Bass program (same NEFF on all 8 cores)."""
    nc = bacc.Bacc("TRN2", debug=False)

    xq_t = nc.declare_dram_parameter("xq", [D, S], BF16, isOutput=False)
    xk_t = nc.declare_dram_parameter("xk", [D, S], BF16, isOutput=False)
    xv_t = nc.declare_dram_parameter("xv", [D, S], BF16, isOutput=False)
    wq_t = nc.declare_dram_parameter("wq", [P, KO * DQ], BF16, isOutput=False)
    wk_t = nc.declare_dram_parameter("wk", [P, KO * DQ], BF16, isOutput=False)
    wv_t = nc.declare_dram_parameter("wv", [P, KO * DQ], BF16, isOutput=False)
    wo_t = nc.declare_dram_parameter("wo", [P, MQ * D], BF16, isOutput=False)
    bq_t = nc.declare_dram_parameter("bq", [P, MQ], F32, isOutput=False)
    bk_t = nc.declare_dram_parameter("bk", [P, MQ], F32, isOutput=False)
    bv_t = nc.declare_dram_parameter("bv", [P, DQ], F32, isOutput=False)
    keep_t = nc.declare_dram_parameter("keep", [S, S], BF16, isOutput=False)
    out_t = nc.declare_dram_parameter("out", [D, S], F16, isOutput=True)

    AF = mybir.ActivationFunctionType
    OP = mybir.AluOpType

    with tile.TileContext(nc) as tc:
        nc.gpsimd.load_library(library_config.attn)
        with (
            tc.tile_pool(name="const", bufs=1) as const,
            tc.tile_pool(name="xs", bufs=3) as xs,
            tc.tile_pool(name="attn", bufs=11) as attnp,
            tc.tile_pool(name="sc", bufs=2) as scp,
            tc.tile_pool(name="outp", bufs=3) as outp,
            tc.tile_pool(name="ps_s", bufs=2, space="PSUM") as ps_s,
            tc.tile_pool(name="ps_c", bufs=1, space="PSUM") as ps_c,
            tc.tile_pool(name="ps_o", bufs=2, space="PSUM") as ps_o,
        ):
            # ---- persistent SBUF tensors ----
            wq_sb = const.tile([P, KO, DQ], BF16, tag="wq")
            wk_sb = const.tile([P, KO, DQ], BF16, tag="wk")
            wv_sb = const.tile([P, KO, DQ], BF16, tag="wv")
            wo_sb = const.tile([P, MQ, D], BF16, tag="wo")
            bq_sb = const.tile([P, MQ], F32, tag="bq")
            bk_sb = const.tile([P, MQ], F32, tag="bk")
            bv_sb = const.tile([P, DQ], F32, tag="bv")
            qT_sb = const.tile([P, MQ, S], BF16, tag="qT")
            kT_sb = const.tile([P, MQ, S], BF16, tag="kT")
            v_sb = const.tile([P, SKN, NH * 65], BF16, tag="v")
            keep_sb = const.tile([P, SKN, S], BF16, tag="keep")
            ctxT_sb = const.tile([P, MQ, S], BF16, tag="ctxT")
            xa_sb = const.tile([P, KO, S], BF16, tag="xa")  # resident xv->xq
            warm = const.tile([1, 8], F32, tag="warm")
            wmm = const.tile([P, 512], BF16, tag="wmm")

            # preload the exp table set on ScalarE while stage A runs
            nc.vector.memset(warm, 0.0)
            nc.scalar.activation(warm, warm, AF.Exp, scale=1.0)
            nc.vector.memset(wmm, 0.0)

            pswarm = ps_o.tile([P, 512], F32, tag="o")

            def warm_burst(lhs, rhs, n=10, dst=None):
                """Dummy matmuls keeping the PE HAM clock-gate open.  Full
                128x128 stationary operand: skinny matmuls don't register
                enough array activity for the HAM to stay at K=8/8."""
                w = rhs.shape[-1]
                if dst is None:
                    dst = pswarm
                for _ in range(n):
                    nc.tensor.matmul(dst[:, 0:w], lhs,
                                     rhs, start=True, stop=True)

            warm_burst(wmm[:, 0:128], wmm[:, 0:512], 16)

            # ones column per head in the V tile (softmax denominator trick)
            v_strided = v_sb[:].rearrange("p s (h c) -> p s h c", h=NH)
            nc.vector.memset(v_strided[:, :, :, 64:65], 1.0)

            def psum_slots():
                """8 independent [128,512] accumulation slots spanning all
                psum pools (stage A only; stage B owns the pools then)."""
                t0 = ps_s.tile([P, SQC], F32, tag="s")
                t1 = ps_s.tile([P, SQC], F32, tag="s")
                tcx = ps_c.tile([P, SQC], F32, tag="c")
                o0 = ps_o.tile([P, 512], F32, tag="o")
                o1 = ps_o.tile([P, 512], F32, tag="o")
                return [t0[:, 0:512], t0[:, 512:1024],
                        t1[:, 0:512], t1[:, 512:1024],
                        tcx[:, 0:512], tcx[:, 512:1024],
                        o0[:, :], o1[:, :]]

            def dma_keep(c, sqh):
                nc.sync.dma_start(
                    keep_sb[:, c, sqh * SQC:(sqh + 1) * SQC],
                    keep_t[c * P:(c + 1) * P, sqh * SQC:(sqh + 1) * SQC])

            # ---- stage A ----
            # K projection: stream xk, prefetch xv halves into resident xa
            nc.sync.dma_start(wk_sb, wk_t[:].rearrange("p (ko m) -> p ko m", ko=KO))
            nc.sync.dma_start(bk_sb, bk_t[:])
            # dep on wk DMA: covers the DMA-latency head before K's matmuls
            warm_burst(wk_sb[:, 0, 0:128], wmm[:, 0:512], 6)

            slots = psum_slots()
            for kk in range(KO):
                t = xs.tile([P, S], BF16, tag="xt")
                nc.sync.dma_start(t, xk_t[kk * P:(kk + 1) * P, :])
                for g in range(8):
                    m, n = g // 4, g % 4
                    nc.tensor.matmul(
                        slots[g],
                        wk_sb[:, kk, m * P:(m + 1) * P],
                        t[:, n * 512:(n + 1) * 512],
                        start=(kk == 0), stop=(kk == KO - 1),
                    )
                    if kk == KO - 1:
                        nc.vector.tensor_scalar_add(
                            kT_sb[:, m, n * 512:(n + 1) * 512],
                            slots[g], bk_sb[:, m:m + 1])

            nc.sync.dma_start(wv_sb, wv_t[:].rearrange("p (ko m) -> p ko m", ko=KO))
            nc.sync.dma_start(bv_sb, bv_t[:])
            # xv first halves stream just-in-time into wave0
            for kk in range(KO):
                nc.sync.dma_start(xa_sb[:, kk, 0:SQC],
                                  xv_t[kk * P:(kk + 1) * P, 0:SQC])
            nc.sync.dma_start(wq_sb, wq_t[:].rearrange("p (ko m) -> p ko m", ko=KO))
            nc.sync.dma_start(bq_sb, bq_t[:])

            # xq parks in keep_sb's sqh1 region (written much later; Tile's
            # WAR tracking hands the space back to the real mask chunks)
            def xq_ap(kk, n):
                c = 2 * kk + n // 2
                o = SQC + (n % 2) * 512
                return keep_sb[:, c, o:o + 512]

            # V projection from resident xa; second xv half + xq ride along
            for w0 in (0, 8):
                slots = psum_slots()
                for kk in range(KO):
                    if w0 == 0:
                        nc.sync.dma_start(xa_sb[:, kk, SQC:S],
                                          xv_t[kk * P:(kk + 1) * P, SQC:S])
                        nc.sync.dma_start(
                            keep_sb[:, 2 * kk, SQC:S],
                            xq_t[kk * P:(kk + 1) * P, 0:SQC])
                    for g in range(8):
                        sv = w0 + g
                        nc.tensor.matmul(
                            slots[g][:, 0:DQ],
                            xa_sb[:, kk, sv * P:(sv + 1) * P],
                            wv_sb[:, kk, :],
                            start=(kk == 0), stop=(kk == KO - 1),
                        )
                        if kk == KO - 1:
                            nc.vector.tensor_tensor(
                                v_strided[:, sv, :, 0:64],
                                slots[g][:, 0:DQ].rearrange(
                                    "p (h c) -> p h c", h=NH),
                                bv_sb[:].rearrange("p (h c) -> p h c", h=NH),
                                OP.add,
                            )
                    if w0 == 8:
                        nc.sync.dma_start(
                            keep_sb[:, 2 * kk + 1, SQC:S],
                            xq_t[kk * P:(kk + 1) * P, SQC:S])

            # Q projection, hm=0 half (heads 0,1) — stage B starts after this
            t0 = ps_s.tile([P, SQC], F32, tag="s")
            t1 = ps_s.tile([P, SQC], F32, tag="s")
            qslots = [t0[:, 0:512], t0[:, 512:1024],
                      t1[:, 0:512], t1[:, 512:1024]]
            for kk in range(KO):
                for n in range(4):
                    nc.tensor.matmul(
                        qslots[n],
                        wq_sb[:, kk, 0:P],
                        xq_ap(kk, n),
                        start=(kk == 0), stop=(kk == KO - 1),
                    )
                    if kk == KO - 1:
                        nc.vector.tensor_scalar_add(
                            qT_sb[:, 0, n * 512:(n + 1) * 512],
                            qslots[n], bq_sb[:, 0:1])

            # mask chunks + wo (ride DMA during early stage B, after xq);
            # sqh1 chunks are emitted after the first sweep, once the Q
            # fillers' reads of the parked xq exist for WAR ordering
            for c in range(SKN):
                dma_keep(c, 0)
            nc.sync.dma_start(wo_sb, wo_t[:].rearrange("p (mq n) -> p mq n", mq=MQ))

            # Q hm=1 half as filler closures, injected into sweep(h0)
            qm1_state = {}

            def qm1_filler(sub, kk):
                if kk == 0:
                    qm1_state[sub] = [
                        ps_o.tile([P, 512], F32, name=f"qm1_{sub}_{i}", tag="o")
                        for i in range(2)]
                tiles = qm1_state[sub]
                for i in range(2):
                    n = sub * 2 + i
                    nc.tensor.matmul(
                        tiles[i],
                        wq_sb[:, kk, P:2 * P],
                        xq_ap(kk, n),
                        start=(kk == 0), stop=(kk == KO - 1),
                    )
                    if kk == KO - 1:
                        nc.vector.tensor_scalar_add(
                            qT_sb[:, 1, n * 512:(n + 1) * 512],
                            tiles[i], bq_sb[:, 1:2])

            fillers = [lambda sub=sub, kk=kk: qm1_filler(sub, kk)
                       for sub in range(2) for kk in range(KO)]

            # ---- stage B: attention, ScalarE-saturated per-head sweeps ----
            def normalize(cps, h, sq0, halves=1):
                """Free cps with one staging copy; normalize lazily.
                HW quirk: custom-DVE / gpsimd ops only work at base partition
                0, so the den row is shifted to partition 0 via SBUF DMA.
                halves=2 pipelines the chain at 512 granularity (short tail)."""
                hb, hm = (h % 2) * 64, h // 2
                w = SQC // halves
                ctxu = scp.tile([65, SQC], F32, tag="ctxu")
                den0 = scp.tile([1, SQC], F32, tag="den0", bufs=1)
                scl = scp.tile([64, SQC], F32, tag="scl", bufs=1)
                cn = scp.tile([64, SQC], BF16, name="cn", tag="cn", bufs=1) if hb else None
                for i in range(halves):
                    s = slice(i * w, (i + 1) * w)
                    so = slice(sq0 + i * w, sq0 + (i + 1) * w)
                    if halves > 1:
                        # tail fast-path: den row first so the recip chain
                        # doesn't wait behind the big staging copy
                        nc.vector.tensor_copy(ctxu[64:65, s], cps[64:65, s])
                        nc.sync.dma_start(den0[:, s], ctxu[64:65, s])
                        nc.vector.tensor_copy(ctxu[0:64, s], cps[0:64, s])
                    else:
                        nc.vector.tensor_copy(ctxu[:, s], cps[0:65, s])
                        nc.sync.dma_start(den0[:, s], ctxu[64:65, s])
                    nc.vector.reciprocal_approx_fast(
                        out=den0[:, s], in_=den0[:, s])
                    nc.gpsimd.partition_broadcast(scl[:, s], den0[0:1, s])
                    if hb == 0:
                        nc.vector.tensor_tensor(
                            ctxT_sb[0:64, hm, so], ctxu[0:64, s], scl[:, s],
                            OP.mult)
                    else:
                        nc.vector.tensor_tensor(
                            cn[:, s], ctxu[0:64, s], scl[:, s], OP.mult)
                        nc.sync.dma_start(ctxT_sb[64:128, hm, so], cn[:, s])

            def oproj_unit(do, n2, sq0, eng="v", ps=None):
                if ps is None:
                    ps = ps_o.tile([P, 512], F32, tag="o")
                for kk in range(MQ):
                    nc.tensor.matmul(
                        ps,
                        wo_sb[:, kk, do * P:(do + 1) * P],
                        ctxT_sb[:, kk, sq0 + n2 * 512:sq0 + (n2 + 1) * 512],
                        start=(kk == 0), stop=(kk == MQ - 1),
                    )
                ot = outp.tile([P, 512], F16, tag="ot")
                if eng == "v":
                    nc.vector.tensor_copy(ot, ps)
                else:
                    nc.scalar.copy(ot, ps)
                nc.sync.dma_start(
                    out_t[do * P:(do + 1) * P,
                          sq0 + n2 * 512:sq0 + (n2 + 1) * 512], ot)

            def scores_for(h, sq0, sk):
                hb, hm = (h % 2) * 64, h // 2
                sps = ps_s.tile([P, SQC], F32, name="sps", tag="s")
                for j in range(2):
                    nc.tensor.matmul(
                        sps[:, j * 512:(j + 1) * 512],
                        kT_sb[hb:hb + 64, hm, sk * P:(sk + 1) * P],
                        qT_sb[hb:hb + 64, hm,
                              sq0 + j * 512:sq0 + (j + 1) * 512],
                        start=True, stop=True,
                    )
                return sps

            def sweep(h, sq0, ounits, fill, final=False, pend=None, nxt=None):
                hb, hm = (h % 2) * 64, h // 2
                cps = ps_c.tile([P, SQC], F32, tag="c")

                def ctx(sk, at):
                    for j in range(2):
                        nc.tensor.matmul(
                            cps[:65, j * 512:(j + 1) * 512],
                            v_sb[:, sk, h * 65:(h + 1) * 65],
                            at[:, j * 512:(j + 1) * 512],
                            start=(sk == 0),
                            stop=(sk == SKN - 1),
                        )

                if not pend:
                    pend = [scores_for(h, sq0, 0), scores_for(h, sq0, 1)]
                nso = []
                ats = []
                for sk in range(SKN):
                    sps = pend.pop(0)
                    at = attnp.tile([P, SQC], BF16, tag="at")
                    nc.scalar.activation(at, sps, AF.Exp, scale=0.125)
                    nc.vector.tensor_tensor(
                        at, at, keep_sb[:, sk, sq0:sq0 + SQC], OP.mult)
                    # PE fillers ride the ramp / idle slack
                    for _ in range(2 if sk < 3 else 1):
                        if fill:
                            fill.pop(0)()
                    if sk + 2 < SKN:
                        pend.append(scores_for(h, sq0, sk + 2))
                    if ounits and sk >= 8 and sk % 2 == 0:
                        oproj_unit(*ounits.pop(0))
                    # next sweep's first scores jump ahead of the final ctx
                    # ops so the exp stream crosses the boundary gap-free
                    if sk == SKN - 1 and nxt:
                        nso.append(scores_for(nxt[0], nxt[1], 0))
                    # ctx trails one step so psum hand-off can't block PE
                    ats.append((sk, at))
                    if len(ats) > 1:
                        ctx(*ats.pop(0))
                if nxt:
                    nso.append(scores_for(nxt[0], nxt[1], 1))
                while ats:
                    ctx(*ats.pop(0))
                normalize(cps, h, sq0, halves=2 if final else 1)
                return nso

            ounits = []
            # odd heads first: the tail-gating last sweeps then write
            # their normalized ctx straight into ctxT (no shift DMA)
            order = [(h, sqh * SQC) for sqh in range(NSQ)
                     for h in (1, 3, 0, 2)]
            pend = None
            for idx, (h, sq0) in enumerate(order):
                nxt = order[idx + 1] if idx + 1 < len(order) else None
                pend = sweep(h, sq0, ounits,
                             fillers if idx == 0 else None,
                             final=(idx == len(order) - 1),
                             pend=pend, nxt=nxt)
                if idx == 0:
                    for c in range(SKN):
                        dma_keep(c, 1)
                if idx % NH == NH - 1:
                    ounits.extend((do, n2, sq0)
                                  for do in range(KO) for n2 in range(2))

            # tail drain: keep the PE warm through the last normalize chain,
            # then rotate units through 6 psum slots (score pool is free now)
            wtail = ps_s.tile([P, SQC], F32, tag="s")
            warm_burst(wq_sb[:, 0, 0:128], qT_sb[:, 0, 0:512], 24, dst=wtail)
            t2 = ps_s.tile([P, SQC], F32, tag="s")
            tail_slots = [wtail[:, 0:512], wtail[:, 512:1024],
                          t2[:, 0:512], t2[:, 512:1024], None, None]
            for i, u in enumerate(ounits):
                oproj_unit(*u, eng="v" if i % 2 else "s",
                           ps=tail_slots[i % 6])

    nc.compile()
    return nc


_NC_CACHE = {}


def _get_nc():
    if "nc" not in _NC_CACHE:
        _NC_CACHE["nc"] = build_nc()
    return _NC_CACHE["nc"]


def _pack_w(wT, ko):
    """[D, M] weight (already transposed) -> [P, ko*M] in SBUF layout."""
    d, m = wT.shape
    return np.ascontiguousarray(
        wT.reshape(ko, P, m).transpose(1, 0, 2).reshape(P, ko * m))


def make_in_maps(query, key, value, mask, Wq, bq, Wk, bk, Wv, bv, Wo, bo):
    """Build the 8 per-core input maps (host-side shard + layout prep)."""
    nb = query.shape[0]
    per_b = []
    for b in range(nb):
        xqT = np.ascontiguousarray(query[b].T).astype(NP_BF16)
        xkT = np.ascontiguousarray(key[b].T).astype(NP_BF16)
        xvT = np.ascontiguousarray(value[b].T).astype(NP_BF16)
        keepT = np.ascontiguousarray((~mask[b, 0]).T).astype(NP_BF16)
        per_b.append((xqT, xkT, xvT, keepT))
    per_g = []
    for g in range(4):
        sl = slice(g * DQ, (g + 1) * DQ)
        per_g.append((
            _pack_w(Wq[sl].T.astype(NP_BF16), KO),
            _pack_w(Wk[sl].T.astype(NP_BF16), KO),
            _pack_w(Wv[sl].T.astype(NP_BF16), KO),
            _pack_w(Wo[:, sl].T.astype(NP_BF16), MQ),
            np.ascontiguousarray(bq[sl].reshape(DQ // P, P).T).astype(np.float32),
            np.ascontiguousarray(bk[sl].reshape(DQ // P, P).T).astype(np.float32),
            np.ascontiguousarray(np.broadcast_to(bv[sl], (P, DQ))).astype(np.float32),
        ))
    in_maps = []
    for c in range(NCORES):
        b, g = c // 4, c % 4
        xqT, xkT, xvT, keepT = per_b[b % nb]
        wqT, wkT, wvT, woT, bq2, bk2, bvr = per_g[g]
        in_maps.append({
            "xq": xqT, "xk": xkT, "xv": xvT,
            "wq": wqT, "wk": wkT, "wv": wvT, "wo": woT,
            "bq": bq2, "bk": bk2, "bv": bvr,
            "keep": keepT,
        })
    return in_maps


def gather_output(results, bo, nb=B, s=S, d=D):
    out = np.empty((nb, s, d), np.float32)
    for b in range(nb):
        acc = results[4 * b]["out"].astype(np.float32)
        for g in range(1, 4):
            acc += results[4 * b + g]["out"].astype(np.float32)
        out[b] = acc.T
    out += bo.astype(np.float32)
    return out


def run_on_cores(in_maps, trace=False, **kw):
    nc = _get_nc()
    return run_bass_kernel_spmd(nc, in_maps, list(range(NCORES)), trace=trace, **kw)


def kernel(query, key, value, mask, Wq, bq, Wk, bk, Wv, bv, Wo, bo):
    in_maps = make_in_maps(query, key, value, mask,
                           Wq, bq, Wk, bk, Wv, bv, Wo, bo)
    res = run_on_cores(in_maps, trace=False)
    return gather_output(res.results, bo)


# revision 66
# speedup vs baseline: 1.3910x; 1.0128x over previous
"""Multi-head attention Bass kernel for Trainium2, sharded over 8 NeuronCores.

Sharding: core c handles batch b = c//4 and head-group g = c%4 (4 of 16 heads,
i.e. a 256-wide slice of the QKV projection output).  Each core computes its
heads' attention and a partial output projection (contribution of its 256
ctx columns to the full [S, D] output).  The host sums the 4 partials per
batch (fp32) and adds the output bias.

Device-side design (v4 — ScalarE-saturated pipeline, HAM kept warm):
  - activations shipped pre-transposed: xT = x.T  [D, S]; weights shipped
    pre-packed in the exact SBUF layout (contiguous DMA).
  - scores computed transposed (scoresT[sk, sq]) so attention weights leave
    softmax with sk on partitions — the contraction layout attn@V needs.
  - softmax denominator comes free from a ones-column appended to V
    (ctx psum row 64 = sum_sk attn);  no max-subtraction (scores bounded).
  - stage B processes one head at a time, sweeping sk chunks with a
    double-buffered score psum: the PE issues scores(sk+2) the moment
    exp(sk) frees a buffer, so the Exp stream on ScalarE (the critical
    engine: ~1 elem/cycle/lane) never waits.  ctx matmuls trail one step
    so psum hand-offs can't head-of-line-block the PE queue.
  - HAM (PE clock gate) management: dummy warm-up matmuls cover the DMA
    latency head; the Q projection is split — its hm=1 half is injected
    as filler matmuls into the first sweep so the PE has no idle window
    at the stage A->B transition and stays at 2.4 GHz.
  - normalize frees ctx-psum with one staging copy and runs the
    recip/broadcast/divide lazily; even heads write straight into ctxT.
  - output projection is emitted one unit per sweep step into the PE's
    idle slack during the NEXT sq block's sweeps; final block drains in a
    short tail with psum->SBUF copies split across Vector/Scalar.
  - psum budget (16KB/part): score 2x[128,1024]f32 + ctx 1x[128,1024]f32
    + oproj 2x[128,512]f32 = 8 banks exactly.  Stage A reuses all four
    pools as 8 independent 512-wide accumulation slots, kk-outer.
"""

import numpy as np
import ml_dtypes

import concourse.bass as bass
import concourse.mybir as mybir
import concourse.tile as tile
from concourse import bacc, library_config
from concourse.bass_utils import run_bass_kernel_spmd

# Problem shapes (hardcoded per contest rules).
B, S, D, H, DH = 2, 2048, 1024, 16, 64
NCORES = 8
NH = 4            # heads per core
DQ = NH * DH      # 256: per-core q/k/v width
P = 128

F32 = mybir.dt.float32
F16 = mybir.dt.float16
BF16 = mybir.dt.bfloat16
NP_BF16 = ml_dtypes.bfloat16

SQC = 1024        # sq block per sweep
NSQ = S // SQC    # 2
SKN = S // P      # 16 sk chunks
KO = D // P       # 8 contraction chunks for projections
MQ = DQ // P      # 2


def build_nc():
    """Build the per-core Bass program (same NEFF on all 8 cores)."""
    nc = bacc.Bacc("TRN2", debug=False)

    xq_t = nc.declare_dram_parameter("xq", [D, S], BF16, isOutput=False)
    xk_t = nc.declare_dram_parameter("xk", [D, S], BF16, isOutput=False)
    xv_t = nc.declare_dram_parameter("xv", [D, S], BF16, isOutput=False)
    wq_t = nc.declare_dram_parameter("wq", [P, KO * DQ], BF16, isOutput=False)
    wk_t = nc.declare_dram_parameter("wk", [P, KO * DQ], BF16, isOutput=False)
    wv_t = nc.declare_dram_parameter("wv", [P, KO * DQ], BF16, isOutput=False)
    wo_t = nc.declare_dram_parameter("wo", [P, MQ * D], BF16, isOutput=False)
    bq_t = nc.declare_dram_parameter("bq", [P, MQ], F32, isOutput=False)
    bk_t = nc.declare_dram_parameter("bk", [P, MQ], F32, isOutput=False)
    bv_t = nc.declare_dram_parameter("bv", [P, DQ], F32, isOutput=False)
    keep_t = nc.declare_dram_parameter("keep", [S, S], BF16, isOutput=False)
    out_t = nc.declare_dram_parameter("out", [D, S], F16, isOutput=True)

    AF = mybir.ActivationFunctionType
    OP = mybir.AluOpType

    with tile.TileContext(nc) as tc:
        nc.gpsimd.load_library(library_config.attn)
        with (
            tc.tile_pool(name="const", bufs=1) as const,
            tc.tile_pool(name="xs", bufs=3) as xs,
            tc.tile_pool(name="attn", bufs=11) as attnp,
            tc.tile_pool(name="sc", bufs=2) as scp,
            tc.tile_pool(name="outp", bufs=3) as outp,
            tc.tile_pool(name="ps_s", bufs=2, space="PSUM") as ps_s,
            tc.tile_pool(name="ps_c", bufs=1, space="PSUM") as ps_c,
            tc.tile_pool(name="ps_o", bufs=2, space="PSUM") as ps_o,
        ):
            # ---- persistent SBUF tensors ----
            wq_sb = const.tile([P, KO, DQ], BF16, tag="wq")
            wk_sb = const.tile([P, KO, DQ], BF16, tag="wk")
            wv_sb = const.tile([P, KO, DQ], BF16, tag="wv")
            wo_sb = const.tile([P, MQ, D], BF16, tag="wo")
            bq_sb = const.tile([P, MQ], F32, tag="bq")
            bk_sb = const.tile([P, MQ], F32, tag="bk")
            bv_sb = const.tile([P, DQ], F32, tag="bv")
            qT_sb = const.tile([P, MQ, S], BF16, tag="qT")
            kT_sb = const.tile([P, MQ, S], BF16, tag="kT")
            v_sb = const.tile([P, SKN, NH * 65], BF16, tag="v")
            keep_sb = const.tile([P, SKN, S], BF16, tag="keep")
            ctxT_sb = const.tile([P, MQ, S], BF16, tag="ctxT")
            xa_sb = const.tile([P, KO, S], BF16, tag="xa")  # resident xv->xq
            warm = const.tile([1, 8], F32, tag="warm")
            wmm = const.tile([P, 512], BF16, tag="wmm")

            # preload the exp table set on ScalarE while stage A runs
            nc.vector.memset(warm, 0.0)
            nc.scalar.activation(warm, warm, AF.Exp, scale=1.0)
            nc.vector.memset(wmm, 0.0)

            pswarm = ps_o.tile([P, 512], F32, tag="o")

            def warm_burst(lhs, rhs, n=10, dst=None):
                """Dummy matmuls keeping the PE HAM clock-gate open.  Full
                128x128 stationary operand: skinny matmuls don't register
                enough array activity for the HAM to stay at K=8/8."""
                w = rhs.shape[-1]
                if dst is None:
                    dst = pswarm
                for _ in range(n):
                    nc.tensor.matmul(dst[:, 0:w], lhs,
                                     rhs, start=True, stop=True)

            warm_burst(wmm[:, 0:128], wmm[:, 0:512], 16)

            # ones column per head in the V tile (softmax denominator trick)
            v_strided = v_sb[:].rearrange("p s (h c) -> p s h c", h=NH)
            nc.vector.memset(v_strided[:, :, :, 64:65], 1.0)

            def psum_slots():
                """8 independent [128,512] accumulation slots spanning all
                psum pools (stage A only; stage B owns the pools then)."""
                t0 = ps_s.tile([P, SQC], F32, tag="s")
                t1 = ps_s.tile([P, SQC], F32, tag="s")
                tcx = ps_c.tile([P, SQC], F32, tag="c")
                o0 = ps_o.tile([P, 512], F32, tag="o")
                o1 = ps_o.tile([P, 512], F32, tag="o")
                return [t0[:, 0:512], t0[:, 512:1024],
                        t1[:, 0:512], t1[:, 512:1024],
                        tcx[:, 0:512], tcx[:, 512:1024],
                        o0[:, :], o1[:, :]]

            def dma_keep(c, sqh):
                nc.sync.dma_start(
                    keep_sb[:, c, sqh * SQC:(sqh + 1) * SQC],
                    keep_t[c * P:(c + 1) * P, sqh * SQC:(sqh + 1) * SQC])

            # ---- stage A ----
            # K projection: stream xk, prefetch xv halves into resident xa
            nc.sync.dma_start(wk_sb, wk_t[:].rearrange("p (ko m) -> p ko m", ko=KO))
            nc.sync.dma_start(bk_sb, bk_t[:])
            # dep on wk DMA: covers the DMA-latency head before K's matmuls
            warm_burst(wk_sb[:, 0, 0:128], wmm[:, 0:512], 6)

            slots = psum_slots()
            for kk in range(KO):
                t = xs.tile([P, S], BF16, tag="xt")
                nc.sync.dma_start(t, xk_t[kk * P:(kk + 1) * P, :])
                for g in range(8):
                    m, n = g // 4, g % 4
                    nc.tensor.matmul(
                        slots[g],
                        wk_sb[:, kk, m * P:(m + 1) * P],
                        t[:, n * 512:(n + 1) * 512],
                        start=(kk == 0), stop=(kk == KO - 1),
                    )
                    if kk == KO - 1:
                        nc.vector.tensor_scalar_add(
                            kT_sb[:, m, n * 512:(n + 1) * 512],
                            slots[g], bk_sb[:, m:m + 1])

            nc.sync.dma_start(wv_sb, wv_t[:].rearrange("p (ko m) -> p ko m", ko=KO))
            nc.sync.dma_start(bv_sb, bv_t[:])
            # xv first halves stream just-in-time into wave0
            for kk in range(KO):
                nc.sync.dma_start(xa_sb[:, kk, 0:SQC],
                                  xv_t[kk * P:(kk + 1) * P, 0:SQC])
            nc.sync.dma_start(wq_sb, wq_t[:].rearrange("p (ko m) -> p ko m", ko=KO))
            nc.sync.dma_start(bq_sb, bq_t[:])

            # xq parks in keep_sb's sqh1 region (written much later; Tile's
            # WAR tracking hands the space back to the real mask chunks)
            def xq_ap(kk, n):
                c = 2 * kk + n // 2
                o = SQC + (n % 2) * 512
                return keep_sb[:, c, o:o + 512]

            # V projection from resident xa; second xv half + xq ride along
            for w0 in (0, 8):
                slots = psum_slots()
                for kk in range(KO):
                    if w0 == 0:
                        nc.sync.dma_start(xa_sb[:, kk, SQC:S],
                                          xv_t[kk * P:(kk + 1) * P, SQC:S])
                        nc.sync.dma_start(
                            keep_sb[:, 2 * kk, SQC:S],
                            xq_t[kk * P:(kk + 1) * P, 0:SQC])
                    for g in range(8):
                        sv = w0 + g
                        nc.tensor.matmul(
                            slots[g][:, 0:DQ],
                            xa_sb[:, kk, sv * P:(sv + 1) * P],
                            wv_sb[:, kk, :],
                            start=(kk == 0), stop=(kk == KO - 1),
                        )
                        if kk == KO - 1:
                            nc.vector.tensor_tensor(
                                v_strided[:, sv, :, 0:64],
                                slots[g][:, 0:DQ].rearrange(
                                    "p (h c) -> p h c", h=NH),
                                bv_sb[:].rearrange("p (h c) -> p h c", h=NH),
                                OP.add,
                            )
                    if w0 == 8:
                        nc.sync.dma_start(
                            keep_sb[:, 2 * kk + 1, SQC:S],
                            xq_t[kk * P:(kk + 1) * P, SQC:S])

            # Q projection, hm=0 half (heads 0,1) — stage B starts after this
            t0 = ps_s.tile([P, SQC], F32, tag="s")
            t1 = ps_s.tile([P, SQC], F32, tag="s")
            qslots = [t0[:, 0:512], t0[:, 512:1024],
                      t1[:, 0:512], t1[:, 512:1024]]
            for kk in range(KO):
                for n in range(4):
                    nc.tensor.matmul(
                        qslots[n],
                        wq_sb[:, kk, 0:P],
                        xq_ap(kk, n),
                        start=(kk == 0), stop=(kk == KO - 1),
                    )
                    if kk == KO - 1:
                        nc.vector.tensor_scalar_add(
                            qT_sb[:, 0, n * 512:(n + 1) * 512],
                            qslots[n], bq_sb[:, 0:1])

            # mask chunks + wo (ride DMA during early stage B, after xq);
            # sqh1 chunks are emitted after the first sweep, once the Q
            # fillers' reads of the parked xq exist for WAR ordering
            for c in range(SKN):
                dma_keep(c, 0)
            nc.sync.dma_start(wo_sb, wo_t[:].rearrange("p (mq n) -> p mq n", mq=MQ))

            # Q hm=1 half as filler closures, injected into sweep(h0)
            qm1_state = {}

            def qm1_filler(sub, kk):
                if kk == 0:
                    qm1_state[sub] = [
                        ps_o.tile([P, 512], F32, name=f"qm1_{sub}_{i}", tag="o")
                        for i in range(2)]
                tiles = qm1_state[sub]
                for i in range(2):
                    n = sub * 2 + i
                    nc.tensor.matmul(
                        tiles[i],
                        wq_sb[:, kk, P:2 * P],
                        xq_ap(kk, n),
                        start=(kk == 0), stop=(kk == KO - 1),
                    )
                    if kk == KO - 1:
                        nc.vector.tensor_scalar_add(
                            qT_sb[:, 1, n * 512:(n + 1) * 512],
                            tiles[i], bq_sb[:, 1:2])

            fillers = [lambda sub=sub, kk=kk: qm1_filler(sub, kk)
                       for sub in range(2) for kk in range(KO)]

            # ---- stage B: attention, ScalarE-saturated per-head sweeps ----
            def normalize(cps, h, sq0, halves=1):
                """Free cps with one staging copy; normalize lazily.
                HW quirk: custom-DVE / gpsimd ops only work at base partition
                0, so the den row is shifted to partition 0 via SBUF DMA.
                halves=2 pipelines the chain at 512 granularity (short tail)."""
                hb, hm = (h % 2) * 64, h // 2
                w = SQC // halves
                ctxu = scp.tile([65, SQC], F32, tag="ctxu")
                den0 = scp.tile([1, SQC], F32, tag="den0", bufs=1)
                scl = scp.tile([64, SQC], F32, tag="scl", bufs=1)
                cn = scp.tile([64, SQC], BF16, name="cn", tag="cn", bufs=1) if hb else None
                for i in range(halves):
                    s = slice(i * w, (i + 1) * w)
                    so = slice(sq0 + i * w, sq0 + (i + 1) * w)
                    if halves > 1:
                        # tail fast-path: den row first so the recip chain
                        # doesn't wait behind the big staging copy
                        nc.vector.tensor_copy(ctxu[64:65, s], cps[64:65, s])
                        nc.sync.dma_start(den0[:, s], ctxu[64:65, s])
                        nc.vector.tensor_copy(ctxu[0:64, s], cps[0:64, s])
                    else:
                        nc.vector.tensor_copy(ctxu[:, s], cps[0:65, s])
                        nc.sync.dma_start(den0[:, s], ctxu[64:65, s])
                    nc.vector.reciprocal_approx_fast(
                        out=den0[:, s], in_=den0[:, s])
                    nc.gpsimd.partition_broadcast(scl[:, s], den0[0:1, s])
                    if hb == 0:
                        nc.vector.tensor_tensor(
                            ctxT_sb[0:64, hm, so], ctxu[0:64, s], scl[:, s],
                            OP.mult)
                    else:
                        nc.vector.tensor_tensor(
                            cn[:, s], ctxu[0:64, s], scl[:, s], OP.mult)
                        nc.sync.dma_start(ctxT_sb[64:128, hm, so], cn[:, s])

            def oproj_unit(do, n2, sq0, eng="v", ps=None):
                if ps is None:
                    ps = ps_o.tile([P, 512], F32, tag="o")
                for kk in range(MQ):
                    nc.tensor.matmul(
                        ps,
                        wo_sb[:, kk, do * P:(do + 1) * P],
                        ctxT_sb[:, kk, sq0 + n2 * 512:sq0 + (n2 + 1) * 512],
                        start=(kk == 0), stop=(kk == MQ - 1),
                    )
                ot = outp.tile([P, 512], F16, tag="ot")
                if eng == "v":
                    nc.vector.tensor_copy(ot, ps)
                else:
                    nc.scalar.copy(ot, ps)
                nc.sync.dma_start(
                    out_t[do * P:(do + 1) * P,
                          sq0 + n2 * 512:sq0 + (n2 + 1) * 512], ot)

            def scores_for(h, sq0, sk):
                hb, hm = (h % 2) * 64, h // 2
                sps = ps_s.tile([P, SQC], F32, name="sps", tag="s")
                for j in range(2):
                    nc.tensor.matmul(
                        sps[:, j * 512:(j + 1) * 512],
                        kT_sb[hb:hb + 64, hm, sk * P:(sk + 1) * P],
                        qT_sb[hb:hb + 64, hm,
                              sq0 + j * 512:sq0 + (j + 1) * 512],
                        start=True, stop=True,
                    )
                return sps

            def sweep(h, sq0, ounits, fill, final=False, pend=None, nxt=None,
                      ramp=False, pnorm=None):
                hb, hm = (h % 2) * 64, h // 2
                # allocated lazily: the pool hands buffers over in emission
                # order, so the alloc must come after the deferred normalize
                # of the previous sweep has emitted its reads
                cpsl = []

                def ctx(sk, at):
                    if not cpsl:
                        cpsl.append(ps_c.tile([P, SQC], F32, name="cps",
                                              tag="c"))
                    cps = cpsl[0]
                    for j in range(2):
                        nc.tensor.matmul(
                            cps[:65, j * 512:(j + 1) * 512],
                            v_sb[:, sk, h * 65:(h + 1) * 65],
                            at[:, j * 512:(j + 1) * 512],
                            start=(sk == 0),
                            stop=(sk == SKN - 1),
                        )

                if not pend:
                    pend = [scores_for(h, sq0, 0), scores_for(h, sq0, 1)]
                nso = []
                ats = []
                for sk in range(SKN):
                    sps = pend.pop(0)
                    at = attnp.tile([P, SQC], BF16, tag="at")
                    nc.scalar.activation(at, sps, AF.Exp, scale=0.125)
                    nc.vector.tensor_tensor(
                        at, at, keep_sb[:, sk, sq0:sq0 + SQC], OP.mult)
                    if sk == 0 and pnorm:
                        # previous sweep's normalize, deferred so its DVE
                        # ops don't delay this sweep's boundary mask-mults
                        pnorm()
                        pnorm = None
                    # PE fillers ride the ramp / idle slack (half density
                    # past the ramp: a filler period exceeds the exp cadence)
                    for _ in range(2 if (ramp and sk < 2) else (sk % 2)):
                        if fill:
                            fill.pop(0)()
                    if sk + 2 < SKN:
                        pend.append(scores_for(h, sq0, sk + 2))
                    if ounits and sk >= 8 and sk % 2 == 0:
                        oproj_unit(*ounits.pop(0))
                    # next sweep's first scores jump ahead of the final ctx
                    # ops so the exp stream crosses the boundary gap-free
                    if sk == SKN - 1 and nxt:
                        nso.append(scores_for(nxt[0], nxt[1], 0))
                    # ctx trails one step so psum hand-off can't block PE
                    ats.append((sk, at))
                    if len(ats) > 1:
                        ctx(*ats.pop(0))
                if nxt:
                    nso.append(scores_for(nxt[0], nxt[1], 1))
                while ats:
                    ctx(*ats.pop(0))
                cps = cpsl[0]
                if final:
                    normalize(cps, h, sq0, halves=2)
                    return nso, None
                return nso, (lambda: normalize(cps, h, sq0))

            ounits = []
            # odd heads first: the tail-gating last sweeps then write
            # their normalized ctx straight into ctxT (no shift DMA)
            order = [(h, sqh * SQC) for sqh in range(NSQ)
                     for h in (1, 0, 3, 2)]
            pend, pnorm = None, None
            for idx, (h, sq0) in enumerate(order):
                nxt = order[idx + 1] if idx + 1 < len(order) else None
                pend, pnorm = sweep(h, sq0, ounits,
                                    fillers if idx < 2 else None,
                                    final=(idx == len(order) - 1),
                                    pend=pend, nxt=nxt, ramp=(idx == 0),
                                    pnorm=pnorm)
                if idx == 1:
                    # after the last Q fillers' reads of the parked xq
                    for c in range(SKN):
                        dma_keep(c, 1)
                if idx % NH == NH - 1:
                    ounits.extend((do, n2, sq0)
                                  for do in range(KO) for n2 in range(2))

            # tail drain: keep the PE warm through the last normalize chain,
            # then rotate units through 6 psum slots (score pool is free now)
            wtail = ps_s.tile([P, SQC], F32, tag="s")
            warm_burst(wq_sb[:, 0, 0:128], qT_sb[:, 0, 0:512], 24, dst=wtail)
            t2 = ps_s.tile([P, SQC], F32, tag="s")
            tail_slots = [wtail[:, 0:512], wtail[:, 512:1024],
                          t2[:, 0:512], t2[:, 512:1024], None, None]
            for i, u in enumerate(ounits):
                oproj_unit(*u, eng="v" if i % 2 else "s",
                           ps=tail_slots[i % 6])

    nc.compile()
    return nc


_NC_CACHE = {}


def _get_nc():
    if "nc" not in _NC_CACHE:
        _NC_CACHE["nc"] = build_nc()
    return _NC_CACHE["nc"]


def _pack_w(wT, ko):
    """[D, M] weight (already transposed) -> [P, ko*M] in SBUF layout."""
    d, m = wT.shape
    return np.ascontiguousarray(
        wT.reshape(ko, P, m).transpose(1, 0, 2).reshape(P, ko * m))


def make_in_maps(query, key, value, mask, Wq, bq, Wk, bk, Wv, bv, Wo, bo):
    """Build the 8 per-core input maps (host-side shard + layout prep)."""
    nb = query.shape[0]
    per_b = []
    for b in range(nb):
        xqT = np.ascontiguousarray(query[b].T).astype(NP_BF16)
        xkT = np.ascontiguousarray(key[b].T).astype(NP_BF16)
        xvT = np.ascontiguousarray(value[b].T).astype(NP_BF16)
        keepT = np.ascontiguousarray((~mask[b, 0]).T).astype(NP_BF16)
        per_b.append((xqT, xkT, xvT, keepT))
    per_g = []
    for g in range(4):
        sl = slice(g * DQ, (g + 1) * DQ)
        per_g.append((
            _pack_w(Wq[sl].T.astype(NP_BF16), KO),
            _pack_w(Wk[sl].T.astype(NP_BF16), KO),
            _pack_w(Wv[sl].T.astype(NP_BF16), KO),
            _pack_w(Wo[:, sl].T.astype(NP_BF16), MQ),
            np.ascontiguousarray(bq[sl].reshape(DQ // P, P).T).astype(np.float32),
            np.ascontiguousarray(bk[sl].reshape(DQ // P, P).T).astype(np.float32),
            np.ascontiguousarray(np.broadcast_to(bv[sl], (P, DQ))).astype(np.float32),
        ))
    in_maps = []
    for c in range(NCORES):
        b, g = c // 4, c % 4
        xqT, xkT, xvT, keepT = per_b[b % nb]
        wqT, wkT, wvT, woT, bq2, bk2, bvr = per_g[g]
        in_maps.append({
            "xq": xqT, "xk": xkT, "xv": xvT,
            "wq": wqT, "wk": wkT, "wv": wvT, "wo": woT,
            "bq": bq2, "bk": bk2, "bv": bvr,
            "keep": keepT,
        })
    return in_maps


def gather_output(results, bo, nb=B, s=S, d=D):
    out = np.empty((nb, s, d), np.float32)
    for b in range(nb):
        acc = results[4 * b]["out"].astype(np.float32)
        for g in range(1, 4):
            acc += results[4 * b + g]["out"].astype(np.float32)
        out[b] = acc.T
    out += bo.astype(np.float32)
    return out


def run_on_cores(in_maps, trace=False, **kw):
    nc = _get_nc()
    return run_bass_kernel_spmd(nc, in_maps, list(range(NCORES)), trace=trace, **kw)


def kernel(query, key, value, mask, Wq, bq, Wk, bk, Wv, bv, Wo, bo):
    in_maps = make_in_maps(query, key, value, mask,
                           Wq, bq, Wk, bk, Wv, bv, Wo, bo)
    res = run_on_cores(in_maps, trace=False)
    return gather_output(res.results, bo)
